# revision 10
# baseline (speedup 1.0000x reference)
"""Multi-head attention kernel for Trainium2, 8 NeuronCores.

Problem: B=4, T=2048, D=1024, H=16 heads, head_dim=64.
Sharding: core c -> batch b = c//2, head group g = c%2 (8 heads each).
Each core computes QKV projections for its 512 features and full
attention for its 8 heads over its batch. No cross-core communication.

Per-core layout (all matmul inputs bf16, fp32 accumulation):
  - x is passed transposed+chunked: xt[p, dc, t] = x[b, t, 128*dc+p]
  - weights passed chunked:  wq[p, dc, f] = Wq[128*dc+p, 512*g+f]
  - Q^T/K^T computed feature-major [feat, t] so attention scores
    S^T[k, q] = sum_d K^T[d, k] Q^T[d, q] come out with k on partitions
  - V computed in natural [t, f] layout, augmented with a ones column:
    PV matmul accumulates [65, 512] where row 64 = softmax denominator
  - softmax needs no max subtraction: |S/8| <= ~7 for N(0,1) inputs
  - output written per head as O^T [64, t]; host transposes/concats
"""

import os
import sys

for _p in ("/opt/trn_rl_repo", "/opt/pypackages"):
    if _p not in sys.path:
        sys.path.insert(0, _p)

import numpy as np
import ml_dtypes

B, T, D, H = 4, 2048, 1024, 16
HD = D // H            # 64 head dim
N_CORES = 8
G = 2                  # head groups (cores per batch)
F = D // G             # 512 features per core
HPC = H // G           # 8 heads per core
P = 128
DC = D // P            # 8 contraction chunks
NPAIR = HPC // 2       # 4 head pairs per core
QC = 512               # query-chunk (columns per score matmul)
NQC = T // QC          # 4 query chunks
NKT = T // P           # 16 key tiles

BF16 = ml_dtypes.bfloat16

_compiled = None  # (nc,) cached across calls in one process


def _build():
    import concourse.bass as bass
    import concourse.tile as tile
    from concourse import bacc, mybir

    fp32 = mybir.dt.float32
    bf16 = mybir.dt.bfloat16
    Exp = mybir.ActivationFunctionType.Exp

    nc = bacc.Bacc("TRN2", target_bir_lowering=False, debug=False,
                   num_devices=N_CORES)

    xt = nc.dram_tensor("xt", [P, DC, T], bf16, kind="ExternalInput").ap()
    wq = nc.dram_tensor("wq", [P, DC, F], bf16, kind="ExternalInput").ap()
    wk = nc.dram_tensor("wk", [P, DC, F], bf16, kind="ExternalInput").ap()
    wv = nc.dram_tensor("wv", [P, DC, F], bf16, kind="ExternalInput").ap()
    bq = nc.dram_tensor("bq", [P, NPAIR], fp32, kind="ExternalInput").ap()
    bk = nc.dram_tensor("bk", [P, NPAIR], fp32, kind="ExternalInput").ap()
    bv = nc.dram_tensor("bv", [P, F], fp32, kind="ExternalInput").ap()
    o = nc.dram_tensor("o", [HPC, HD, T], fp32, kind="ExternalOutput").ap()

    with tile.TileContext(nc) as tc:
        with (
            tc.tile_pool(name="singles", bufs=1) as singles,
            tc.tile_pool(name="es", bufs=4) as es_pool,
            tc.tile_pool(name="stage", bufs=2) as stage_pool,
            tc.tile_pool(name="norm", bufs=2) as norm_pool,
            tc.tile_pool(name="sps", bufs=2, space="PSUM") as sps_pool,
            tc.tile_pool(name="pv", bufs=1, space="PSUM") as pv_pool,
            tc.tile_pool(name="qkv", bufs=2, space="PSUM") as qkv_pool,
        ):
            # ---- persistent SBUF tensors ----
            xt_sb = singles.tile([P, DC, T], bf16, tag="xt")
            wq_sb = singles.tile([P, DC, F], bf16, tag="wq")
            wk_sb = singles.tile([P, DC, F], bf16, tag="wk")
            wv_sb = singles.tile([P, DC, F], bf16, tag="wv")
            bq_sb = singles.tile([P, NPAIR], fp32, tag="bq")
            bk_sb = singles.tile([P, NPAIR], fp32, tag="bk")
            bv_sb = singles.tile([P, F], fp32, tag="bv")
            # per-pair Q^T/K^T [feat-in-pair, t] and V [t-in-ktile, kt, hp, 65]
            qt_sb = [singles.tile([P, T], bf16, tag=f"qt{j}", name=f"qt{j}")
                     for j in range(NPAIR)]
            kt_sb = [singles.tile([P, T], bf16, tag=f"kt{j}", name=f"kt{j}")
                     for j in range(NPAIR)]
            v_sb = [singles.tile([P, NKT, 2, HD + 1], bf16, tag=f"v{j}",
                                 name=f"v{j}")
                    for j in range(NPAIR)]
            # normalize staging, separate per head-slot (a/b):
            # rzs holds 1/Z on partition 64, rz0 the same row moved to
            # partition 0 (gpsimd cross-partition copy), rzb the broadcast
            rzs = [singles.tile([HD + 1, QC], fp32, tag=f"rzs{i}",
                                name=f"rzs{i}") for i in range(2)]
            rz0 = [singles.tile([1, QC], fp32, tag=f"rz0{i}",
                                name=f"rz0{i}") for i in range(2)]
            rzb = [singles.tile([HD, QC], fp32, tag=f"rzb{i}",
                                name=f"rzb{i}") for i in range(2)]

            nc.sync.dma_start(out=xt_sb[:], in_=xt[:])
            nc.sync.dma_start(out=wq_sb[:], in_=wq[:])
            nc.sync.dma_start(out=wk_sb[:], in_=wk[:])
            nc.sync.dma_start(out=wv_sb[:], in_=wv[:])
            nc.sync.dma_start(out=bq_sb[:], in_=bq[:])
            nc.sync.dma_start(out=bk_sb[:], in_=bk[:])
            nc.sync.dma_start(out=bv_sb[:], in_=bv[:])
            for j in range(NPAIR):
                nc.vector.memset(v_sb[j][:, :, :, HD:HD + 1], 1.0)

            def emit_qk_proj(j):
                """Q^T/K^T rows for pair j: psum [f=128, t=512] per t-chunk."""
                for w_sb, b_sb, dst in ((wq_sb, bq_sb, qt_sb[j]),
                                        (wk_sb, bk_sb, kt_sb[j])):
                    for tcn in range(T // 512):
                        ps = qkv_pool.tile([P, 512], fp32, tag="qkv")
                        for dc in range(DC):
                            nc.tensor.matmul(
                                ps[:],
                                w_sb[:, dc, P * j:P * (j + 1)],
                                xt_sb[:, dc, 512 * tcn:512 * (tcn + 1)],
                                start=(dc == 0), stop=(dc == DC - 1),
                            )
                        nc.vector.tensor_scalar_add(
                            out=dst[:, 512 * tcn:512 * (tcn + 1)],
                            in0=ps[:],
                            scalar1=b_sb[:, j:j + 1],
                        )

            def emit_v_proj(tt_lo, tt_hi):
                """V rows, all pairs at once: psum [t=128, f=512] per t-tile."""
                for tt in range(tt_lo, tt_hi):
                    ps = qkv_pool.tile([P, F], fp32, tag="qkv")
                    for dc in range(DC):
                        nc.tensor.matmul(
                            ps[:],
                            xt_sb[:, dc, P * tt:P * (tt + 1)],
                            wv_sb[:, dc, :],
                            start=(dc == 0), stop=(dc == DC - 1),
                        )
                    for j in range(NPAIR):
                        nc.vector.tensor_add(
                            out=v_sb[j][:, tt, :, 0:HD],
                            in0=ps[:, P * j:P * (j + 1)].rearrange(
                                "p (h d) -> p h d", h=2),
                            in1=bv_sb[:, P * j:P * (j + 1)].rearrange(
                                "p (h d) -> p h d", h=2),
                        )

            emit_qk_proj(0)
            emit_v_proj(0, NKT)

            for j in range(NPAIR):
                qt, kt, vv = qt_sb[j], kt_sb[j], v_sb[j]
                for qc in range(NQC):
                    q0 = QC * qc
                    pva = pv_pool.tile([HD + 1, QC], fp32, tag="pva")
                    pvb = pv_pool.tile([HD + 1, QC], fp32, tag="pvb")
                    if qc == 0:
                        sta = stage_pool.tile([HD, T], fp32, tag="sta")
                        stb = stage_pool.tile([HD, T], fp32, tag="stb")
                    for g in range(NKT // 2):
                        kt0, kt1 = 2 * g, 2 * g + 1
                        sA = sps_pool.tile([P, 2, QC], fp32, tag="sps")
                        sB = sps_pool.tile([P, 2, QC], fp32, tag="sps")
                        # scores S^T[k, q]; A on PE rows 0-63, B on 64-127,
                        # interleaved so the row-disjoint matmuls overlap
                        for i, ktn in enumerate((kt0, kt1)):
                            for hp, s in ((0, sA), (1, sB)):
                                nc.tensor.matmul(
                                    s[:, i, :],
                                    kt[HD * hp:HD * (hp + 1),
                                       P * ktn:P * (ktn + 1)],
                                    qt[HD * hp:HD * (hp + 1), q0:q0 + QC],
                                    start=True, stop=True,
                                )
                        esA = es_pool.tile([P, 2, QC], bf16, tag="es")
                        esB = es_pool.tile([P, 2, QC], bf16, tag="es")
                        nc.scalar.activation(
                            esA[:].rearrange("p a b -> p (a b)"),
                            sA[:].rearrange("p a b -> p (a b)"),
                            Exp, scale=0.125)
                        nc.scalar.activation(
                            esB[:].rearrange("p a b -> p (a b)"),
                            sB[:].rearrange("p a b -> p (a b)"),
                            Exp, scale=0.125)
                        for i, ktn in enumerate((kt0, kt1)):
                            first = ktn == 0
                            last = ktn == NKT - 1
                            nc.tensor.matmul(
                                pva[:], vv[:, ktn, 0, :], esA[:, i, :],
                                start=first, stop=last)
                            nc.tensor.matmul(
                                pvb[:], vv[:, ktn, 1, :], esB[:, i, :],
                                start=first, stop=last)
                    # normalize: row HD of pv holds Z = sum_k exp(s/8).
                    # Copy psum->sbuf first so the PV banks free up fast
                    # (the recip/broadcast chain is slow but off-critical).
                    for hp, pv_t, st in ((0, pva, sta), (1, pvb, stb)):
                        pvc = norm_pool.tile([HD + 1, QC], fp32,
                                             tag=f"pvc{hp}", name=f"pvc{hp}")
                        nc.vector.tensor_copy(pvc[:], pv_t[:])
                        nc.vector.reciprocal(rzs[hp][HD:HD + 1, :],
                                             pvc[HD:HD + 1, :])
                        # Z sits on partition 64; partition_broadcast only
                        # reads partition 0 on HW, so move it there first
                        nc.gpsimd.tensor_copy(rz0[hp][:],
                                              rzs[hp][HD:HD + 1, :])
                        nc.gpsimd.partition_broadcast(rzb[hp][:], rz0[hp][:])
                        nc.vector.tensor_mul(st[:, q0:q0 + QC],
                                             pvc[0:HD, :], rzb[hp][:])
                    # feed the PE pipeline with next pair's projections
                    if j + 1 < NPAIR:
                        if NQC >= 4:
                            if qc == 0:
                                emit_qk_proj(j + 1)
                        elif qc == 0:
                            emit_qk_proj(j + 1)
                    if qc == NQC - 1:
                        nc.sync.dma_start(out=o[2 * j], in_=sta[:])
                        nc.sync.dma_start(out=o[2 * j + 1], in_=stb[:])

    nc.compile()
    return nc


def _prep_inputs(x, Wq, bq, Wk, bk, Wv, bv):
    """Host-side shard + layout prep. Returns per-core input dicts."""
    in_maps = []
    xt_cache = {}
    w_cache = {}
    for c in range(N_CORES):
        b, g = c // G, c % G
        if b not in xt_cache:
            xtb = np.ascontiguousarray(x[b].T).astype(BF16)      # [D, T]
            xt_cache[b] = np.ascontiguousarray(
                xtb.reshape(DC, P, T).transpose(1, 0, 2))        # [P, DC, T]
        if g not in w_cache:
            def _w(W):
                Wg = W[:, F * g:F * (g + 1)].astype(BF16)        # [D, F]
                return np.ascontiguousarray(
                    Wg.reshape(DC, P, F).transpose(1, 0, 2))     # [P, DC, F]
            bqg = bq[F * g:F * (g + 1)].astype(np.float32)
            bkg = bk[F * g:F * (g + 1)].astype(np.float32)
            bvg = bv[F * g:F * (g + 1)].astype(np.float32)
            w_cache[g] = {
                "wq": _w(Wq), "wk": _w(Wk), "wv": _w(Wv),
                # [P, NPAIR]: bias for feature 128*j + p
                "bq": np.ascontiguousarray(bqg.reshape(NPAIR, P).T),
                "bk": np.ascontiguousarray(bkg.reshape(NPAIR, P).T),
                # [P, F]: broadcast along partitions
                "bv": np.ascontiguousarray(
                    np.broadcast_to(bvg[None, :], (P, F))),
            }
        in_maps.append({"xt": xt_cache[b], **w_cache[g]})
    return in_maps


def _run(in_maps, trace_dir=None, trace_cores=None):
    from concourse.bass_utils import run_bass_kernel_spmd

    global _compiled
    if _compiled is None:
        _compiled = _build()
    nc = _compiled

    if trace_dir is not None:
        from trn_agent_boot.trn_boot import _ntff_profile_via_ctypes
        hook = _ntff_profile_via_ctypes("/opt/axon/libaxon_pjrt.so")
        with hook(trace_dir, trace_cores):
            res = run_bass_kernel_spmd(nc, in_maps,
                                       core_ids=list(range(N_CORES)))
    else:
        res = run_bass_kernel_spmd(nc, in_maps, core_ids=list(range(N_CORES)))
    return res


def kernel(x, Wq, bq, Wk, bk, Wv, bv, _trace_dir=None, _trace_cores=None):
    x = np.asarray(x, dtype=np.float32)
    in_maps = _prep_inputs(x, np.asarray(Wq), np.asarray(bq), np.asarray(Wk),
                           np.asarray(bk), np.asarray(Wv), np.asarray(bv))
    res = _run(in_maps, _trace_dir, _trace_cores)
    out = np.empty((B, T, D), np.float32)
    for c in range(N_CORES):
        b, g = c // G, c % G
        oc = np.asarray(res.results[c]["o"])          # [HPC, HD, T]
        out[b, :, F * g:F * (g + 1)] = (
            oc.transpose(2, 0, 1).reshape(T, F))
    return out


# revision 14
# speedup vs baseline: 1.3057x; 1.3057x over previous
"""Multi-head attention kernel for Trainium2, 8 NeuronCores.

Problem: B=4, T=2048, D=1024, H=16 heads, head_dim=64.
Sharding: core c -> batch b = c//2, head group g = c%2 (8 heads each).
Each core computes QKV projections for its 512 features and full
attention for its 8 heads over its batch. No cross-core communication.

Per-core layout (all matmul inputs bf16, fp32 accumulation):
  - x is passed transposed+chunked: xt[p, dc, t] = x[b, t, 128*dc+p]
  - weights passed chunked:  wq[p, dc, f] = Wq[128*dc+p, 512*g+f]
  - Q^T/K^T computed feature-major [feat, t] so attention scores
    S^T[k, q] = sum_d K^T[d, k] Q^T[d, q] come out with k on partitions
  - V computed in natural [t, f] layout, augmented with a ones column:
    PV matmul accumulates [65, 512] where row 64 = softmax denominator
  - softmax needs no max subtraction: |S/8| <= ~7 for N(0,1) inputs
  - output written per head as O^T [64, t]; host transposes/concats
"""

import os
import sys

for _p in ("/opt/trn_rl_repo", "/opt/pypackages"):
    if _p not in sys.path:
        sys.path.insert(0, _p)

import numpy as np
import ml_dtypes

B, T, D, H = 4, 2048, 1024, 16
HD = D // H            # 64 head dim
N_CORES = 8
G = 2                  # head groups (cores per batch)
F = D // G             # 512 features per core
HPC = H // G           # 8 heads per core
P = 128
DC = D // P            # 8 contraction chunks
NPAIR = HPC // 2       # 4 head pairs per core
QC = 512               # query-chunk (columns per score matmul)
NQC = T // QC          # 4 query chunks
NKT = T // P           # 16 key tiles

BF16 = ml_dtypes.bfloat16

_compiled = None  # (nc,) cached across calls in one process


def _build():
    import concourse.bass as bass
    import concourse.tile as tile
    from concourse import bacc, mybir

    fp32 = mybir.dt.float32
    bf16 = mybir.dt.bfloat16
    Exp = mybir.ActivationFunctionType.Exp

    nc = bacc.Bacc("TRN2", target_bir_lowering=False, debug=False,
                   num_devices=N_CORES)

    xt = nc.dram_tensor("xt", [P, DC, T], bf16, kind="ExternalInput").ap()
    wq = nc.dram_tensor("wq", [P, DC, F], bf16, kind="ExternalInput").ap()
    wk = nc.dram_tensor("wk", [P, DC, F], bf16, kind="ExternalInput").ap()
    wv = nc.dram_tensor("wv", [P, DC, F], bf16, kind="ExternalInput").ap()
    bq = nc.dram_tensor("bq", [P, NPAIR], fp32, kind="ExternalInput").ap()
    bk = nc.dram_tensor("bk", [P, NPAIR], fp32, kind="ExternalInput").ap()
    bv = nc.dram_tensor("bv", [P, F], fp32, kind="ExternalInput").ap()
    o = nc.dram_tensor("o", [HPC, HD, T], fp32, kind="ExternalOutput").ap()

    with tile.TileContext(nc) as tc:
        with (
            tc.tile_pool(name="singles", bufs=1) as singles,
            tc.tile_pool(name="es", bufs=4) as es_pool,
            tc.tile_pool(name="stage", bufs=2) as stage_pool,
            tc.tile_pool(name="norm", bufs=2) as norm_pool,
            tc.tile_pool(name="sps", bufs=2, space="PSUM") as sps_pool,
            tc.tile_pool(name="pv", bufs=1, space="PSUM") as pv_pool,
            tc.tile_pool(name="qkv", bufs=2, space="PSUM") as qkv_pool,
        ):
            # ---- persistent SBUF tensors ----
            xt_sb = singles.tile([P, DC, T], bf16, tag="xt")
            wq_sb = singles.tile([P, DC, F], bf16, tag="wq")
            wk_sb = singles.tile([P, DC, F], bf16, tag="wk")
            wv_sb = singles.tile([P, DC, F], bf16, tag="wv")
            bq_sb = singles.tile([P, NPAIR], fp32, tag="bq")
            bk_sb = singles.tile([P, NPAIR], fp32, tag="bk")
            bv_sb = singles.tile([P, F], fp32, tag="bv")
            # per-pair Q^T/K^T [feat-in-pair, t] and V [t-in-ktile, kt, hp, 65]
            qt_sb = [singles.tile([P, T], bf16, tag=f"qt{j}", name=f"qt{j}")
                     for j in range(NPAIR)]
            kt_sb = [singles.tile([P, T], bf16, tag=f"kt{j}", name=f"kt{j}")
                     for j in range(NPAIR)]
            v_sb = [singles.tile([P, NKT, 2, HD + 1], bf16, tag=f"v{j}",
                                 name=f"v{j}")
                    for j in range(NPAIR)]
            # normalize staging, separate per head-slot (a/b):
            # rzs holds 1/Z on partition 64, rz0 the same row moved to
            # partition 0 (gpsimd cross-partition copy), rzb the broadcast
            rzs = [singles.tile([HD + 1, QC], fp32, tag=f"rzs{i}",
                                name=f"rzs{i}") for i in range(2)]
            rz0 = [singles.tile([1, QC], fp32, tag=f"rz0{i}",
                                name=f"rz0{i}") for i in range(2)]
            rzb = [singles.tile([HD, QC], fp32, tag=f"rzb{i}",
                                name=f"rzb{i}") for i in range(2)]

            nc.sync.dma_start(out=xt_sb[:], in_=xt[:])
            nc.sync.dma_start(out=wq_sb[:], in_=wq[:])
            nc.sync.dma_start(out=wk_sb[:], in_=wk[:])
            nc.sync.dma_start(out=wv_sb[:], in_=wv[:])
            nc.sync.dma_start(out=bq_sb[:], in_=bq[:])
            nc.sync.dma_start(out=bk_sb[:], in_=bk[:])
            nc.sync.dma_start(out=bv_sb[:], in_=bv[:])
            for j in range(NPAIR):
                nc.vector.memset(v_sb[j][:, :, :, HD:HD + 1], 1.0)

            def emit_qk_chunk(j, which, tcn):
                """One [f=128, t=512] t-chunk of Q^T or K^T for pair j."""
                w_sb, b_sb, dst = ((wq_sb, bq_sb, qt_sb[j]) if which == "q"
                                   else (wk_sb, bk_sb, kt_sb[j]))
                ps = qkv_pool.tile([P, 512], fp32, tag="qkv", name="qkps")
                for dc in range(DC):
                    nc.tensor.matmul(
                        ps[:],
                        w_sb[:, dc, P * j:P * (j + 1)],
                        xt_sb[:, dc, 512 * tcn:512 * (tcn + 1)],
                        start=(dc == 0), stop=(dc == DC - 1),
                    )
                nc.vector.tensor_scalar_add(
                    out=dst[:, 512 * tcn:512 * (tcn + 1)],
                    in0=ps[:],
                    scalar1=b_sb[:, j:j + 1],
                )

            def emit_qk_proj(j):
                for which in ("q", "k"):
                    for tcn in range(T // 512):
                        emit_qk_chunk(j, which, tcn)

            def emit_v_proj(tt_lo, tt_hi):
                """V rows, all pairs at once: psum [t=128, f=512] per t-tile."""
                for tt in range(tt_lo, tt_hi):
                    ps = qkv_pool.tile([P, F], fp32, tag="qkv")
                    for dc in range(DC):
                        nc.tensor.matmul(
                            ps[:],
                            xt_sb[:, dc, P * tt:P * (tt + 1)],
                            wv_sb[:, dc, :],
                            start=(dc == 0), stop=(dc == DC - 1),
                        )
                    for j in range(NPAIR):
                        nc.vector.tensor_add(
                            out=v_sb[j][:, tt, :, 0:HD],
                            in0=ps[:, P * j:P * (j + 1)].rearrange(
                                "p (h d) -> p h d", h=2),
                            in1=bv_sb[:, P * j:P * (j + 1)].rearrange(
                                "p (h d) -> p h d", h=2),
                        )

            emit_qk_proj(0)
            emit_v_proj(0, NKT)

            for j in range(NPAIR):
                qt, kt, vv = qt_sb[j], kt_sb[j], v_sb[j]
                for qc in range(NQC):
                    q0 = QC * qc
                    pva = pv_pool.tile([HD + 1, QC], fp32, tag="pva")
                    pvb = pv_pool.tile([HD + 1, QC], fp32, tag="pvb")
                    if qc == 0:
                        sta = stage_pool.tile([HD, T], fp32, tag="sta")
                        stb = stage_pool.tile([HD, T], fp32, tag="stb")
                    for g in range(NKT // 2):
                        kt0, kt1 = 2 * g, 2 * g + 1
                        # keep the PE warm through ACT-paced stretches: the
                        # last pair has no projection filler left, so issue
                        # tiny throwaway matmuls (HAM re-throttles the PE
                        # clock after ~3.4us of contiguous idle)
                        if j == NPAIR - 1:
                            dm = qkv_pool.tile([P, 256], fp32, tag="qkv",
                                               name="warmmm")
                            nc.tensor.matmul(
                                dm[:], wq_sb[:, 0, 0:P],
                                xt_sb[:, 0, 0:256], start=True, stop=True)
                        sA = sps_pool.tile([P, 2, QC], fp32, tag="sps")
                        sB = sps_pool.tile([P, 2, QC], fp32, tag="sps")
                        # scores S^T[k, q]; A on PE rows 0-63, B on 64-127,
                        # interleaved so the row-disjoint matmuls overlap
                        for i, ktn in enumerate((kt0, kt1)):
                            for hp, s in ((0, sA), (1, sB)):
                                nc.tensor.matmul(
                                    s[:, i, :],
                                    kt[HD * hp:HD * (hp + 1),
                                       P * ktn:P * (ktn + 1)],
                                    qt[HD * hp:HD * (hp + 1), q0:q0 + QC],
                                    start=True, stop=True,
                                )
                        esA = es_pool.tile([P, 2, QC], bf16, tag="es")
                        esB = es_pool.tile([P, 2, QC], bf16, tag="es")
                        nc.scalar.activation(
                            esA[:].rearrange("p a b -> p (a b)"),
                            sA[:].rearrange("p a b -> p (a b)"),
                            Exp, scale=0.125)
                        nc.scalar.activation(
                            esB[:].rearrange("p a b -> p (a b)"),
                            sB[:].rearrange("p a b -> p (a b)"),
                            Exp, scale=0.125)
                        for i, ktn in enumerate((kt0, kt1)):
                            first = ktn == 0
                            last = ktn == NKT - 1
                            nc.tensor.matmul(
                                pva[:], vv[:, ktn, 0, :], esA[:, i, :],
                                start=first, stop=last)
                            nc.tensor.matmul(
                                pvb[:], vv[:, ktn, 1, :], esB[:, i, :],
                                start=first, stop=last)
                    # normalize: row HD of pv holds Z = sum_k exp(s/8).
                    # Copy psum->sbuf first so the PV banks free up fast
                    # (the recip/broadcast chain is slow but off-critical).
                    for hp, pv_t, st in ((0, pva, sta), (1, pvb, stb)):
                        pvc = norm_pool.tile([HD + 1, QC], fp32,
                                             tag=f"pvc{hp}", name=f"pvc{hp}")
                        nc.vector.tensor_copy(pvc[:], pv_t[:])
                        nc.vector.reciprocal(rzs[hp][HD:HD + 1, :],
                                             pvc[HD:HD + 1, :])
                        # Z sits on partition 64; partition_broadcast only
                        # reads partition 0 on HW, so move it there first
                        nc.gpsimd.tensor_copy(rz0[hp][:],
                                              rzs[hp][HD:HD + 1, :])
                        nc.gpsimd.partition_broadcast(rzb[hp][:], rz0[hp][:])
                        nc.vector.tensor_mul(st[:, q0:q0 + QC],
                                             pvc[0:HD, :], rzb[hp][:])
                    # feed the PE pipeline with next pair's projections,
                    # spread evenly across this pair's iterations (K first:
                    # the next pair's first scores need all of K^T but only
                    # the first quarter of Q^T)
                    if j + 1 < NPAIR:
                        if NQC >= 4:
                            pieces = [("k", 0), ("k", 1), ("k", 2), ("k", 3),
                                      ("q", 0), ("q", 1), ("q", 2), ("q", 3)]
                            for which, tcn in pieces[2 * qc:2 * qc + 2]:
                                emit_qk_chunk(j + 1, which, tcn)
                        elif qc == 0:
                            emit_qk_proj(j + 1)
                    if qc == NQC - 1:
                        nc.sync.dma_start(out=o[2 * j], in_=sta[:])
                        nc.sync.dma_start(out=o[2 * j + 1], in_=stb[:])

    nc.compile()
    return nc


def _prep_inputs(x, Wq, bq, Wk, bk, Wv, bv):
    """Host-side shard + layout prep. Returns per-core input dicts."""
    in_maps = []
    xt_cache = {}
    w_cache = {}
    for c in range(N_CORES):
        b, g = c // G, c % G
        if b not in xt_cache:
            xtb = np.ascontiguousarray(x[b].T).astype(BF16)      # [D, T]
            xt_cache[b] = np.ascontiguousarray(
                xtb.reshape(DC, P, T).transpose(1, 0, 2))        # [P, DC, T]
        if g not in w_cache:
            def _w(W):
                Wg = W[:, F * g:F * (g + 1)].astype(BF16)        # [D, F]
                return np.ascontiguousarray(
                    Wg.reshape(DC, P, F).transpose(1, 0, 2))     # [P, DC, F]
            bqg = bq[F * g:F * (g + 1)].astype(np.float32)
            bkg = bk[F * g:F * (g + 1)].astype(np.float32)
            bvg = bv[F * g:F * (g + 1)].astype(np.float32)
            w_cache[g] = {
                "wq": _w(Wq), "wk": _w(Wk), "wv": _w(Wv),
                # [P, NPAIR]: bias for feature 128*j + p
                "bq": np.ascontiguousarray(bqg.reshape(NPAIR, P).T),
                "bk": np.ascontiguousarray(bkg.reshape(NPAIR, P).T),
                # [P, F]: broadcast along partitions
                "bv": np.ascontiguousarray(
                    np.broadcast_to(bvg[None, :], (P, F))),
            }
        in_maps.append({"xt": xt_cache[b], **w_cache[g]})
    return in_maps


def _run(in_maps, trace_dir=None, trace_cores=None):
    from concourse.bass_utils import run_bass_kernel_spmd

    global _compiled
    if _compiled is None:
        _compiled = _build()
    nc = _compiled

    if trace_dir is not None:
        from trn_agent_boot.trn_boot import _ntff_profile_via_ctypes
        hook = _ntff_profile_via_ctypes("/opt/axon/libaxon_pjrt.so")
        with hook(trace_dir, trace_cores):
            res = run_bass_kernel_spmd(nc, in_maps,
                                       core_ids=list(range(N_CORES)))
    else:
        res = run_bass_kernel_spmd(nc, in_maps, core_ids=list(range(N_CORES)))
    return res


def kernel(x, Wq, bq, Wk, bk, Wv, bv, _trace_dir=None, _trace_cores=None):
    x = np.asarray(x, dtype=np.float32)
    in_maps = _prep_inputs(x, np.asarray(Wq), np.asarray(bq), np.asarray(Wk),
                           np.asarray(bk), np.asarray(Wv), np.asarray(bv))
    res = _run(in_maps, _trace_dir, _trace_cores)
    out = np.empty((B, T, D), np.float32)
    for c in range(N_CORES):
        b, g = c // G, c % G
        oc = np.asarray(res.results[c]["o"])          # [HPC, HD, T]
        out[b, :, F * g:F * (g + 1)] = (
            oc.transpose(2, 0, 1).reshape(T, F))
    return out


# revision 19
# speedup vs baseline: 1.5672x; 1.2003x over previous
"""Multi-head attention kernel for Trainium2, 8 NeuronCores.

Problem: B=4, T=2048, D=1024, H=16 heads, head_dim=64.
Sharding: core c -> batch b = c//2, head group g = c%2 (8 heads each).
Each core computes QKV projections for its 512 features and full
attention for its 8 heads over its batch. No cross-core communication.

Per-core layout (all matmul inputs bf16, fp32 accumulation):
  - x is passed transposed+chunked: xt[p, dc, t] = x[b, t, 128*dc+p]
  - weights passed chunked:  wq[p, dc, f] = Wq[128*dc+p, 512*g+f]
  - Q^T/K^T computed feature-major [feat, t] so attention scores
    S^T[k, q] = sum_d K^T[d, k] Q^T[d, q] come out with k on partitions
  - V computed in natural [t, f] layout, augmented with a ones column:
    PV matmul accumulates [65, 512] where row 64 = softmax denominator
  - softmax needs no max subtraction: |S/8| <= ~7 for N(0,1) inputs
  - output written per head as O^T [64, t]; host transposes/concats
"""

import os
import sys

for _p in ("/opt/trn_rl_repo", "/opt/pypackages"):
    if _p not in sys.path:
        sys.path.insert(0, _p)

import numpy as np
import ml_dtypes

B, T, D, H = 4, 2048, 1024, 16
HD = D // H            # 64 head dim
N_CORES = 8
G = 2                  # head groups (cores per batch)
F = D // G             # 512 features per core
HPC = H // G           # 8 heads per core
P = 128
DC = D // P            # 8 contraction chunks
NPAIR = HPC // 2       # 4 head pairs per core
QC = 512               # query-chunk (columns per score matmul)
NQC = T // QC          # 4 query chunks
NKT = T // P           # 16 key tiles

BF16 = ml_dtypes.bfloat16

_compiled = None  # (nc,) cached across calls in one process


def _build():
    import concourse.bass as bass
    import concourse.tile as tile
    from concourse import bacc, mybir

    fp32 = mybir.dt.float32
    bf16 = mybir.dt.bfloat16
    Exp = mybir.ActivationFunctionType.Exp

    nc = bacc.Bacc("TRN2", target_bir_lowering=False, debug=False,
                   num_devices=N_CORES)

    xt = nc.dram_tensor("xt", [P, DC, T], bf16, kind="ExternalInput").ap()
    wq = nc.dram_tensor("wq", [P, DC, F], bf16, kind="ExternalInput").ap()
    wk = nc.dram_tensor("wk", [P, DC, F], bf16, kind="ExternalInput").ap()
    wv = nc.dram_tensor("wv", [P, DC, F], bf16, kind="ExternalInput").ap()
    bq = nc.dram_tensor("bq", [P, NPAIR], fp32, kind="ExternalInput").ap()
    bk = nc.dram_tensor("bk", [P, NPAIR], fp32, kind="ExternalInput").ap()
    bv = nc.dram_tensor("bv", [P, F], fp32, kind="ExternalInput").ap()
    o = nc.dram_tensor("o", [HPC, HD, T], fp32, kind="ExternalOutput").ap()

    with tile.TileContext(nc) as tc:
        with (
            tc.tile_pool(name="singles", bufs=1) as singles,
            tc.tile_pool(name="es", bufs=4) as es_pool,
            tc.tile_pool(name="stage", bufs=2) as stage_pool,
            tc.tile_pool(name="norm", bufs=2) as norm_pool,
            tc.tile_pool(name="sps", bufs=2, space="PSUM") as sps_pool,
            tc.tile_pool(name="pv", bufs=1, space="PSUM") as pv_pool,
            tc.tile_pool(name="qkv", bufs=2, space="PSUM") as qkv_pool,
        ):
            # ---- persistent SBUF tensors ----
            xt_sb = singles.tile([P, DC, T], bf16, tag="xt")
            wq_sb = singles.tile([P, DC, F], bf16, tag="wq")
            wk_sb = singles.tile([P, DC, F], bf16, tag="wk")
            wv_sb = singles.tile([P, DC, F], bf16, tag="wv")
            bq_sb = singles.tile([P, NPAIR], fp32, tag="bq")
            bk_sb = singles.tile([P, NPAIR], fp32, tag="bk")
            bv_sb = singles.tile([P, F], fp32, tag="bv")
            # per-pair Q^T/K^T [feat-in-pair, t] and V [t-in-ktile, kt, hp, 65]
            qt_sb = [singles.tile([P, T], bf16, tag=f"qt{j}", name=f"qt{j}")
                     for j in range(NPAIR)]
            kt_sb = [singles.tile([P, T], bf16, tag=f"kt{j}", name=f"kt{j}")
                     for j in range(NPAIR)]
            v_sb = [singles.tile([P, NKT, 2, HD + 1], bf16, tag=f"v{j}",
                                 name=f"v{j}")
                    for j in range(NPAIR)]
            # normalize staging, separate per head-slot (a/b). The [1, 512]
            # Z row would use one DVE lane (3.3us reciprocal), so bounce it
            # through a [128, 4] layout via sb->sb DMA: reciprocal runs on
            # 128 lanes, and the gather-back lands on partition 0 (the only
            # partition gpsimd's partition_broadcast can read on HW).
            zcol = [singles.tile([P, 4], fp32, tag=f"zcol{i}",
                                 name=f"zcol{i}") for i in range(2)]
            rz0 = [singles.tile([1, QC], fp32, tag=f"rz0{i}",
                                name=f"rz0{i}") for i in range(2)]
            rzb = [singles.tile([HD, QC], fp32, tag=f"rzb{i}",
                                name=f"rzb{i}") for i in range(2)]

            nc.sync.dma_start(out=xt_sb[:], in_=xt[:])
            nc.sync.dma_start(out=wq_sb[:], in_=wq[:])
            nc.sync.dma_start(out=wk_sb[:], in_=wk[:])
            nc.sync.dma_start(out=wv_sb[:], in_=wv[:])
            nc.sync.dma_start(out=bq_sb[:], in_=bq[:])
            nc.sync.dma_start(out=bk_sb[:], in_=bk[:])
            nc.sync.dma_start(out=bv_sb[:], in_=bv[:])
            for j in range(NPAIR):
                nc.vector.memset(v_sb[j][:, :, :, HD:HD + 1], 1.0)

            def emit_qk_chunk(j, which, tcn):
                """One [f=128, t=512] t-chunk of Q^T or K^T for pair j."""
                w_sb, b_sb, dst = ((wq_sb, bq_sb, qt_sb[j]) if which == "q"
                                   else (wk_sb, bk_sb, kt_sb[j]))
                ps = qkv_pool.tile([P, 512], fp32, tag="qkv", name="qkps")
                for dc in range(DC):
                    nc.tensor.matmul(
                        ps[:],
                        w_sb[:, dc, P * j:P * (j + 1)],
                        xt_sb[:, dc, 512 * tcn:512 * (tcn + 1)],
                        start=(dc == 0), stop=(dc == DC - 1),
                    )
                nc.vector.tensor_scalar_add(
                    out=dst[:, 512 * tcn:512 * (tcn + 1)],
                    in0=ps[:],
                    scalar1=b_sb[:, j:j + 1],
                )

            def emit_qk_proj(j):
                for which in ("q", "k"):
                    for tcn in range(T // 512):
                        emit_qk_chunk(j, which, tcn)

            def emit_v_proj(tt_lo, tt_hi):
                """V rows, all pairs at once: psum [t=128, f=512] per t-tile."""
                for tt in range(tt_lo, tt_hi):
                    ps = qkv_pool.tile([P, F], fp32, tag="qkv")
                    for dc in range(DC):
                        nc.tensor.matmul(
                            ps[:],
                            xt_sb[:, dc, P * tt:P * (tt + 1)],
                            wv_sb[:, dc, :],
                            start=(dc == 0), stop=(dc == DC - 1),
                        )
                    for j in range(NPAIR):
                        nc.vector.tensor_add(
                            out=v_sb[j][:, tt, :, 0:HD],
                            in0=ps[:, P * j:P * (j + 1)].rearrange(
                                "p (h d) -> p h d", h=2),
                            in1=bv_sb[:, P * j:P * (j + 1)].rearrange(
                                "p (h d) -> p h d", h=2),
                        )

            NTC = T // 512
            # prologue: V for all pairs, all of K^T(0), first chunk of Q^T(0)
            for tcn in range(NTC):
                emit_qk_chunk(0, "k", tcn)
            emit_qk_chunk(0, "q", 0)
            emit_v_proj(0, NKT)

            for j in range(NPAIR):
                qt, kt, vv = qt_sb[j], kt_sb[j], v_sb[j]
                for qc in range(NQC):
                    q0 = QC * qc
                    pva = pv_pool.tile([HD + 1, QC], fp32, tag="pva")
                    pvb = pv_pool.tile([HD + 1, QC], fp32, tag="pvb")
                    if qc == 0:
                        sta = stage_pool.tile([HD, T], fp32, tag="sta")
                        stb = stage_pool.tile([HD, T], fp32, tag="stb")
                    for g in range(NKT // 2):
                        kt0, kt1 = 2 * g, 2 * g + 1
                        # keep the PE warm through ACT-paced stretches: the
                        # last pair has no projection filler left, so issue
                        # tiny throwaway matmuls (HAM re-throttles the PE
                        # clock after ~3.4us of contiguous idle)
                        if j == NPAIR - 1:
                            dm = qkv_pool.tile([P, 256], fp32, tag="qkv",
                                               name="warmmm")
                            nc.tensor.matmul(
                                dm[:], wq_sb[:, 0, 0:P],
                                xt_sb[:, 0, 0:256], start=True, stop=True)
                        sA = sps_pool.tile([P, 2, QC], fp32, tag="sps")
                        sB = sps_pool.tile([P, 2, QC], fp32, tag="sps")
                        # scores S^T[k, q]; A on PE rows 0-63, B on 64-127,
                        # interleaved so the row-disjoint matmuls overlap
                        for i, ktn in enumerate((kt0, kt1)):
                            for hp, s in ((0, sA), (1, sB)):
                                nc.tensor.matmul(
                                    s[:, i, :],
                                    kt[HD * hp:HD * (hp + 1),
                                       P * ktn:P * (ktn + 1)],
                                    qt[HD * hp:HD * (hp + 1), q0:q0 + QC],
                                    start=True, stop=True,
                                )
                        esA = es_pool.tile([P, 2, QC], bf16, tag="es")
                        esB = es_pool.tile([P, 2, QC], bf16, tag="es")
                        nc.scalar.activation(
                            esA[:].rearrange("p a b -> p (a b)"),
                            sA[:].rearrange("p a b -> p (a b)"),
                            Exp, scale=0.125)
                        nc.scalar.activation(
                            esB[:].rearrange("p a b -> p (a b)"),
                            sB[:].rearrange("p a b -> p (a b)"),
                            Exp, scale=0.125)
                        for i, ktn in enumerate((kt0, kt1)):
                            first = ktn == 0
                            last = ktn == NKT - 1
                            nc.tensor.matmul(
                                pva[:], vv[:, ktn, 0, :], esA[:, i, :],
                                start=first, stop=last)
                            nc.tensor.matmul(
                                pvb[:], vv[:, ktn, 1, :], esB[:, i, :],
                                start=first, stop=last)
                    # normalize: row HD of pv holds Z = sum_k exp(s/8).
                    # Copy psum->sbuf first so the PV banks free up fast
                    # (the recip/broadcast chain is slow but off-critical).
                    pvcs = []
                    for hp, pv_t in ((0, pva), (1, pvb)):
                        pvc = norm_pool.tile([HD + 1, QC], fp32,
                                             tag=f"pvc{hp}", name=f"pvc{hp}")
                        nc.vector.tensor_copy(pvc[:], pv_t[:])
                        pvcs.append(pvc)
                    for hp, st in ((0, sta), (1, stb)):
                        pvc = pvcs[hp]
                        nc.sync.dma_start(out=zcol[hp][:],
                                          in_=pvc[HD:HD + 1, :])
                        nc.vector.reciprocal(zcol[hp][:], zcol[hp][:])
                        nc.sync.dma_start(out=rz0[hp][:], in_=zcol[hp][:])
                        nc.gpsimd.partition_broadcast(rzb[hp][:], rz0[hp][:])
                        nc.vector.tensor_mul(st[:, q0:q0 + QC],
                                             pvc[0:HD, :], rzb[hp][:])
                    # feed the PE pipeline with projection filler, spread
                    # over every iteration: next pair's K^T chunk-by-chunk,
                    # this pair's remaining Q^T chunks just before use, and
                    # next pair's first Q^T chunk at the boundary
                    for tcn in range(qc * NTC // NQC, (qc + 1) * NTC // NQC):
                        if j + 1 < NPAIR:
                            emit_qk_chunk(j + 1, "k", tcn)
                    nxt = (qc + 1) * NTC // NQC
                    if nxt < NTC:
                        emit_qk_chunk(j, "q", nxt)
                    elif j + 1 < NPAIR:
                        emit_qk_chunk(j + 1, "q", 0)
                    if qc == NQC - 1:
                        nc.sync.dma_start(out=o[2 * j], in_=sta[:])
                        nc.sync.dma_start(out=o[2 * j + 1], in_=stb[:])

    nc.compile()
    return nc


def _prep_inputs(x, Wq, bq, Wk, bk, Wv, bv):
    """Host-side shard + layout prep. Returns per-core input dicts."""
    in_maps = []
    xt_cache = {}
    w_cache = {}
    for c in range(N_CORES):
        b, g = c // G, c % G
        if b not in xt_cache:
            xtb = np.ascontiguousarray(x[b].T).astype(BF16)      # [D, T]
            xt_cache[b] = np.ascontiguousarray(
                xtb.reshape(DC, P, T).transpose(1, 0, 2))        # [P, DC, T]
        if g not in w_cache:
            def _w(W):
                Wg = W[:, F * g:F * (g + 1)].astype(BF16)        # [D, F]
                return np.ascontiguousarray(
                    Wg.reshape(DC, P, F).transpose(1, 0, 2))     # [P, DC, F]
            bqg = bq[F * g:F * (g + 1)].astype(np.float32)
            bkg = bk[F * g:F * (g + 1)].astype(np.float32)
            bvg = bv[F * g:F * (g + 1)].astype(np.float32)
            w_cache[g] = {
                "wq": _w(Wq), "wk": _w(Wk), "wv": _w(Wv),
                # [P, NPAIR]: bias for feature 128*j + p
                "bq": np.ascontiguousarray(bqg.reshape(NPAIR, P).T),
                "bk": np.ascontiguousarray(bkg.reshape(NPAIR, P).T),
                # [P, F]: broadcast along partitions
                "bv": np.ascontiguousarray(
                    np.broadcast_to(bvg[None, :], (P, F))),
            }
        in_maps.append({"xt": xt_cache[b], **w_cache[g]})
    return in_maps


def _run(in_maps, trace_dir=None, trace_cores=None):
    from concourse.bass_utils import run_bass_kernel_spmd

    global _compiled
    if _compiled is None:
        _compiled = _build()
    nc = _compiled

    if trace_dir is not None:
        from trn_agent_boot.trn_boot import _ntff_profile_via_ctypes
        hook = _ntff_profile_via_ctypes("/opt/axon/libaxon_pjrt.so")
        with hook(trace_dir, trace_cores):
            res = run_bass_kernel_spmd(nc, in_maps,
                                       core_ids=list(range(N_CORES)))
    else:
        res = run_bass_kernel_spmd(nc, in_maps, core_ids=list(range(N_CORES)))
    return res


def kernel(x, Wq, bq, Wk, bk, Wv, bv, _trace_dir=None, _trace_cores=None):
    x = np.asarray(x, dtype=np.float32)
    in_maps = _prep_inputs(x, np.asarray(Wq), np.asarray(bq), np.asarray(Wk),
                           np.asarray(bk), np.asarray(Wv), np.asarray(bv))
    res = _run(in_maps, _trace_dir, _trace_cores)
    out = np.empty((B, T, D), np.float32)
    for c in range(N_CORES):
        b, g = c // G, c % G
        oc = np.asarray(res.results[c]["o"])          # [HPC, HD, T]
        out[b, :, F * g:F * (g + 1)] = (
            oc.transpose(2, 0, 1).reshape(T, F))
    return out


# revision 20
# speedup vs baseline: 1.7147x; 1.0941x over previous
"""Multi-head attention kernel for Trainium2, 8 NeuronCores.

Problem: B=4, T=2048, D=1024, H=16 heads, head_dim=64.
Sharding: core c -> batch b = c//2, head group g = c%2 (8 heads each).
Each core computes QKV projections for its 512 features and full
attention for its 8 heads over its batch. No cross-core communication.

Per-core layout (all matmul inputs bf16, fp32 accumulation):
  - x is passed transposed+chunked: xt[p, dc, t] = x[b, t, 128*dc+p]
  - weights passed chunked:  wq[p, dc, f] = Wq[128*dc+p, 512*g+f]
  - Q^T/K^T computed feature-major [feat, t] so attention scores
    S^T[k, q] = sum_d K^T[d, k] Q^T[d, q] come out with k on partitions
  - V computed in natural [t, f] layout, augmented with a ones column:
    PV matmul accumulates [65, 512] where row 64 = softmax denominator
  - softmax needs no max subtraction: |S/8| <= ~7 for N(0,1) inputs
  - output written per head as O^T [64, t]; host transposes/concats
"""

import os
import sys

for _p in ("/opt/trn_rl_repo", "/opt/pypackages"):
    if _p not in sys.path:
        sys.path.insert(0, _p)

import numpy as np
import ml_dtypes

B, T, D, H = 4, 2048, 1024, 16
HD = D // H            # 64 head dim
N_CORES = 8
G = 2                  # head groups (cores per batch)
F = D // G             # 512 features per core
HPC = H // G           # 8 heads per core
P = 128
DC = D // P            # 8 contraction chunks
NPAIR = HPC // 2       # 4 head pairs per core
QC = 512               # query-chunk (columns per score matmul)
NQC = T // QC          # 4 query chunks
NKT = T // P           # 16 key tiles

BF16 = ml_dtypes.bfloat16

_compiled = None  # (nc,) cached across calls in one process


def _build():
    import concourse.bass as bass
    import concourse.tile as tile
    from concourse import bacc, mybir

    fp32 = mybir.dt.float32
    bf16 = mybir.dt.bfloat16
    Exp = mybir.ActivationFunctionType.Exp

    nc = bacc.Bacc("TRN2", target_bir_lowering=False, debug=False,
                   num_devices=N_CORES)

    xt = nc.dram_tensor("xt", [P, DC, T], bf16, kind="ExternalInput").ap()
    wq = nc.dram_tensor("wq", [P, DC, F], bf16, kind="ExternalInput").ap()
    wk = nc.dram_tensor("wk", [P, DC, F], bf16, kind="ExternalInput").ap()
    wv = nc.dram_tensor("wv", [P, DC, F], bf16, kind="ExternalInput").ap()
    bq = nc.dram_tensor("bq", [P, NPAIR], fp32, kind="ExternalInput").ap()
    bk = nc.dram_tensor("bk", [P, NPAIR], fp32, kind="ExternalInput").ap()
    bv = nc.dram_tensor("bv", [P, F], fp32, kind="ExternalInput").ap()
    o = nc.dram_tensor("o", [HPC, HD, T], fp32, kind="ExternalOutput").ap()

    with tile.TileContext(nc) as tc:
        with (
            tc.tile_pool(name="singles", bufs=1) as singles,
            tc.tile_pool(name="es", bufs=4) as es_pool,
            tc.tile_pool(name="stage", bufs=2) as stage_pool,
            tc.tile_pool(name="norm", bufs=2) as norm_pool,
            tc.tile_pool(name="sps", bufs=2, space="PSUM") as sps_pool,
            tc.tile_pool(name="pv", bufs=1, space="PSUM") as pv_pool,
            tc.tile_pool(name="qkv", bufs=2, space="PSUM") as qkv_pool,
        ):
            # ---- persistent SBUF tensors ----
            xt_sb = singles.tile([P, DC, T], bf16, tag="xt")
            wq_sb = singles.tile([P, DC, F], bf16, tag="wq")
            wk_sb = singles.tile([P, DC, F], bf16, tag="wk")
            wv_sb = singles.tile([P, DC, F], bf16, tag="wv")
            bq_sb = singles.tile([P, NPAIR], fp32, tag="bq")
            bk_sb = singles.tile([P, NPAIR], fp32, tag="bk")
            bv_sb = singles.tile([P, F], fp32, tag="bv")
            # per-pair Q^T/K^T [feat-in-pair, t] and V [t-in-ktile, kt, hp, 65]
            qt_sb = [singles.tile([P, T], bf16, tag=f"qt{j}", name=f"qt{j}")
                     for j in range(NPAIR)]
            kt_sb = [singles.tile([P, T], bf16, tag=f"kt{j}", name=f"kt{j}")
                     for j in range(NPAIR)]
            v_sb = [singles.tile([P, NKT, 2, HD + 1], bf16, tag=f"v{j}",
                                 name=f"v{j}")
                    for j in range(NPAIR)]
            # normalize staging, separate per head-slot (a/b). The [1, 512]
            # Z row would use one DVE lane (3.3us reciprocal), so bounce it
            # through a [128, 4] layout via sb->sb DMA: reciprocal runs on
            # 128 lanes, and the gather-back lands on partition 0 (the only
            # partition gpsimd's partition_broadcast can read on HW).
            zcol = [singles.tile([P, 4], fp32, tag=f"zcol{i}",
                                 name=f"zcol{i}") for i in range(2)]
            rz0 = [singles.tile([1, QC], fp32, tag=f"rz0{i}",
                                name=f"rz0{i}") for i in range(2)]
            rzb = [singles.tile([HD, QC], fp32, tag=f"rzb{i}",
                                name=f"rzb{i}") for i in range(2)]

            nc.sync.dma_start(out=xt_sb[:], in_=xt[:])
            nc.sync.dma_start(out=wq_sb[:], in_=wq[:])
            nc.sync.dma_start(out=wk_sb[:], in_=wk[:])
            nc.sync.dma_start(out=wv_sb[:], in_=wv[:])
            nc.sync.dma_start(out=bq_sb[:], in_=bq[:])
            nc.sync.dma_start(out=bk_sb[:], in_=bk[:])
            nc.sync.dma_start(out=bv_sb[:], in_=bv[:])
            for j in range(NPAIR):
                nc.vector.memset(v_sb[j][:, :, :, HD:HD + 1], 1.0)

            def emit_qk_chunk(j, which, tcn):
                """One [f=128, t=512] t-chunk of Q^T or K^T for pair j."""
                w_sb, b_sb, dst = ((wq_sb, bq_sb, qt_sb[j]) if which == "q"
                                   else (wk_sb, bk_sb, kt_sb[j]))
                ps = qkv_pool.tile([P, 512], fp32, tag="qkv", name="qkps")
                for dc in range(DC):
                    nc.tensor.matmul(
                        ps[:],
                        w_sb[:, dc, P * j:P * (j + 1)],
                        xt_sb[:, dc, 512 * tcn:512 * (tcn + 1)],
                        start=(dc == 0), stop=(dc == DC - 1),
                    )
                nc.vector.tensor_scalar_add(
                    out=dst[:, 512 * tcn:512 * (tcn + 1)],
                    in0=ps[:],
                    scalar1=b_sb[:, j:j + 1],
                )

            def emit_qk_proj(j):
                for which in ("q", "k"):
                    for tcn in range(T // 512):
                        emit_qk_chunk(j, which, tcn)

            def emit_v_proj(tt_lo, tt_hi):
                """V rows, all pairs at once: psum [t=128, f=512] per t-tile."""
                for tt in range(tt_lo, tt_hi):
                    ps = qkv_pool.tile([P, F], fp32, tag="qkv")
                    for dc in range(DC):
                        nc.tensor.matmul(
                            ps[:],
                            xt_sb[:, dc, P * tt:P * (tt + 1)],
                            wv_sb[:, dc, :],
                            start=(dc == 0), stop=(dc == DC - 1),
                        )
                    for j in range(NPAIR):
                        nc.vector.tensor_add(
                            out=v_sb[j][:, tt, :, 0:HD],
                            in0=ps[:, P * j:P * (j + 1)].rearrange(
                                "p (h d) -> p h d", h=2),
                            in1=bv_sb[:, P * j:P * (j + 1)].rearrange(
                                "p (h d) -> p h d", h=2),
                        )

            NTC = T // 512
            # prologue: V for all pairs, all of K^T(0), first chunk of Q^T(0)
            for tcn in range(NTC):
                emit_qk_chunk(0, "k", tcn)
            emit_qk_chunk(0, "q", 0)
            emit_v_proj(0, NKT)

            for j in range(NPAIR):
                qt, kt, vv = qt_sb[j], kt_sb[j], v_sb[j]
                for qc in range(NQC):
                    q0 = QC * qc
                    pva = pv_pool.tile([HD + 1, QC], fp32, tag="pva")
                    pvb = pv_pool.tile([HD + 1, QC], fp32, tag="pvb")
                    if qc == 0:
                        sta = stage_pool.tile([HD, T], fp32, tag="sta")
                        stb = stage_pool.tile([HD, T], fp32, tag="stb")
                    for ktn in range(NKT):
                        # keep the PE warm through ACT-paced stretches: the
                        # last pair has no projection filler left, so issue
                        # tiny throwaway matmuls (HAM re-throttles the PE
                        # clock after ~3.4us of contiguous idle)
                        if j == NPAIR - 1 and ktn % 2 == 0:
                            dm = qkv_pool.tile([P, 256], fp32, tag="qkv",
                                               name="warmmm")
                            nc.tensor.matmul(
                                dm[:], wq_sb[:, 0, 0:P],
                                xt_sb[:, 0, 0:256], start=True, stop=True)
                        # scores S^T[k, q] for BOTH heads of the pair in one
                        # 2-bank psum tile: head A on PE rows 0-63, head B
                        # on rows 64-127. Sharing one tile makes the two
                        # matmuls ready simultaneously, so the scheduler
                        # keeps them adjacent and the row-disjoint matmuls
                        # run concurrently on the array (~2x).
                        s = sps_pool.tile([P, 2, QC], fp32, tag="sps",
                                          name="s")
                        for hp in (0, 1):
                            nc.tensor.matmul(
                                s[:, hp, :],
                                kt[HD * hp:HD * (hp + 1),
                                   P * ktn:P * (ktn + 1)],
                                qt[HD * hp:HD * (hp + 1), q0:q0 + QC],
                                start=True, stop=True,
                            )
                        es = es_pool.tile([P, 2, QC], bf16, tag="es",
                                          name="es")
                        nc.scalar.activation(
                            es[:].rearrange("p a b -> p (a b)"),
                            s[:].rearrange("p a b -> p (a b)"),
                            Exp, scale=0.125)
                        first = ktn == 0
                        last = ktn == NKT - 1
                        nc.tensor.matmul(
                            pva[:], vv[:, ktn, 0, :], es[:, 0, :],
                            start=first, stop=last)
                        nc.tensor.matmul(
                            pvb[:], vv[:, ktn, 1, :], es[:, 1, :],
                            start=first, stop=last)
                    # normalize: row HD of pv holds Z = sum_k exp(s/8).
                    # Copy psum->sbuf first so the PV banks free up fast
                    # (the recip/broadcast chain is slow but off-critical).
                    pvcs = []
                    for hp, pv_t in ((0, pva), (1, pvb)):
                        pvc = norm_pool.tile([HD + 1, QC], fp32,
                                             tag=f"pvc{hp}", name=f"pvc{hp}")
                        nc.vector.tensor_copy(pvc[:], pv_t[:])
                        pvcs.append(pvc)
                    for hp, st in ((0, sta), (1, stb)):
                        pvc = pvcs[hp]
                        nc.sync.dma_start(out=zcol[hp][:],
                                          in_=pvc[HD:HD + 1, :])
                        nc.vector.reciprocal(zcol[hp][:], zcol[hp][:])
                        nc.sync.dma_start(out=rz0[hp][:], in_=zcol[hp][:])
                        nc.gpsimd.partition_broadcast(rzb[hp][:], rz0[hp][:])
                        nc.vector.tensor_mul(st[:, q0:q0 + QC],
                                             pvc[0:HD, :], rzb[hp][:])
                    # feed the PE pipeline with projection filler, spread
                    # over every iteration: next pair's K^T chunk-by-chunk,
                    # this pair's remaining Q^T chunks just before use, and
                    # next pair's first Q^T chunk at the boundary
                    for tcn in range(qc * NTC // NQC, (qc + 1) * NTC // NQC):
                        if j + 1 < NPAIR:
                            emit_qk_chunk(j + 1, "k", tcn)
                    nxt = (qc + 1) * NTC // NQC
                    if nxt < NTC:
                        emit_qk_chunk(j, "q", nxt)
                    elif j + 1 < NPAIR:
                        emit_qk_chunk(j + 1, "q", 0)
                    if qc == NQC - 1:
                        nc.sync.dma_start(out=o[2 * j], in_=sta[:])
                        nc.sync.dma_start(out=o[2 * j + 1], in_=stb[:])

    nc.compile()
    return nc


def _prep_inputs(x, Wq, bq, Wk, bk, Wv, bv):
    """Host-side shard + layout prep. Returns per-core input dicts."""
    in_maps = []
    xt_cache = {}
    w_cache = {}
    for c in range(N_CORES):
        b, g = c // G, c % G
        if b not in xt_cache:
            xtb = np.ascontiguousarray(x[b].T).astype(BF16)      # [D, T]
            xt_cache[b] = np.ascontiguousarray(
                xtb.reshape(DC, P, T).transpose(1, 0, 2))        # [P, DC, T]
        if g not in w_cache:
            def _w(W):
                Wg = W[:, F * g:F * (g + 1)].astype(BF16)        # [D, F]
                return np.ascontiguousarray(
                    Wg.reshape(DC, P, F).transpose(1, 0, 2))     # [P, DC, F]
            bqg = bq[F * g:F * (g + 1)].astype(np.float32)
            bkg = bk[F * g:F * (g + 1)].astype(np.float32)
            bvg = bv[F * g:F * (g + 1)].astype(np.float32)
            w_cache[g] = {
                "wq": _w(Wq), "wk": _w(Wk), "wv": _w(Wv),
                # [P, NPAIR]: bias for feature 128*j + p
                "bq": np.ascontiguousarray(bqg.reshape(NPAIR, P).T),
                "bk": np.ascontiguousarray(bkg.reshape(NPAIR, P).T),
                # [P, F]: broadcast along partitions
                "bv": np.ascontiguousarray(
                    np.broadcast_to(bvg[None, :], (P, F))),
            }
        in_maps.append({"xt": xt_cache[b], **w_cache[g]})
    return in_maps


def _run(in_maps, trace_dir=None, trace_cores=None):
    from concourse.bass_utils import run_bass_kernel_spmd

    global _compiled
    if _compiled is None:
        _compiled = _build()
    nc = _compiled

    if trace_dir is not None:
        from trn_agent_boot.trn_boot import _ntff_profile_via_ctypes
        hook = _ntff_profile_via_ctypes("/opt/axon/libaxon_pjrt.so")
        with hook(trace_dir, trace_cores):
            res = run_bass_kernel_spmd(nc, in_maps,
                                       core_ids=list(range(N_CORES)))
    else:
        res = run_bass_kernel_spmd(nc, in_maps, core_ids=list(range(N_CORES)))
    return res


def kernel(x, Wq, bq, Wk, bk, Wv, bv, _trace_dir=None, _trace_cores=None):
    x = np.asarray(x, dtype=np.float32)
    in_maps = _prep_inputs(x, np.asarray(Wq), np.asarray(bq), np.asarray(Wk),
                           np.asarray(bk), np.asarray(Wv), np.asarray(bv))
    res = _run(in_maps, _trace_dir, _trace_cores)
    out = np.empty((B, T, D), np.float32)
    for c in range(N_CORES):
        b, g = c // G, c % G
        oc = np.asarray(res.results[c]["o"])          # [HPC, HD, T]
        out[b, :, F * g:F * (g + 1)] = (
            oc.transpose(2, 0, 1).reshape(T, F))
    return out


# revision 25
# speedup vs baseline: 1.7393x; 1.0143x over previous
"""Multi-head attention kernel for Trainium2, 8 NeuronCores.

Problem: B=4, T=2048, D=1024, H=16 heads, head_dim=64.
Sharding: core c -> batch b = c//2, head group g = c%2 (8 heads each).
Each core computes QKV projections for its 512 features and full
attention for its 8 heads over its batch. No cross-core communication.

Per-core layout (all matmul inputs bf16, fp32 accumulation):
  - x is passed transposed+chunked: xt[p, dc, t] = x[b, t, 128*dc+p]
  - weights passed chunked:  wq[p, dc, f] = Wq[128*dc+p, 512*g+f]
  - Q^T/K^T computed feature-major [feat, t] so attention scores
    S^T[k, q] = sum_d K^T[d, k] Q^T[d, q] come out with k on partitions
  - V computed in natural [t, f] layout, augmented with a ones column:
    PV matmul accumulates [65, 512] where row 64 = softmax denominator
  - softmax needs no max subtraction: |S/8| <= ~7 for N(0,1) inputs
  - output written per head as O^T [64, t]; host transposes/concats
"""

import os
import sys

for _p in ("/opt/trn_rl_repo", "/opt/pypackages"):
    if _p not in sys.path:
        sys.path.insert(0, _p)

import numpy as np
import ml_dtypes

B, T, D, H = 4, 2048, 1024, 16
HD = D // H            # 64 head dim
N_CORES = 8
G = 2                  # head groups (cores per batch)
F = D // G             # 512 features per core
HPC = H // G           # 8 heads per core
P = 128
DC = D // P            # 8 contraction chunks
NPAIR = HPC // 2       # 4 head pairs per core
QC = 512               # query-chunk (columns per score matmul)
NQC = T // QC          # 4 query chunks
NKT = T // P           # 16 key tiles

BF16 = ml_dtypes.bfloat16

_compiled = None  # (nc,) cached across calls in one process


def _build():
    import concourse.bass as bass
    import concourse.tile as tile
    from concourse import bacc, mybir

    fp32 = mybir.dt.float32
    bf16 = mybir.dt.bfloat16
    Exp = mybir.ActivationFunctionType.Exp

    nc = bacc.Bacc("TRN2", target_bir_lowering=False, debug=False,
                   num_devices=N_CORES)

    xt = nc.dram_tensor("xt", [P, DC, T], bf16, kind="ExternalInput").ap()
    wq = nc.dram_tensor("wq", [P, DC, F], bf16, kind="ExternalInput").ap()
    wk = nc.dram_tensor("wk", [P, DC, F], bf16, kind="ExternalInput").ap()
    wv = nc.dram_tensor("wv", [P, DC, F], bf16, kind="ExternalInput").ap()
    bq = nc.dram_tensor("bq", [P, NPAIR], fp32, kind="ExternalInput").ap()
    bk = nc.dram_tensor("bk", [P, NPAIR], fp32, kind="ExternalInput").ap()
    bv = nc.dram_tensor("bv", [P, F], fp32, kind="ExternalInput").ap()
    o = nc.dram_tensor("o", [HPC, HD, T], fp32, kind="ExternalOutput").ap()

    with tile.TileContext(nc) as tc:
        with (
            tc.tile_pool(name="singles", bufs=1) as singles,
            tc.tile_pool(name="es", bufs=20) as es_pool,
            tc.tile_pool(name="stage", bufs=2) as stage_pool,
            tc.tile_pool(name="norm", bufs=2) as norm_pool,
            tc.tile_pool(name="sps", bufs=2, space="PSUM") as sps_pool,
            tc.tile_pool(name="pv", bufs=1, space="PSUM") as pv_pool,
            tc.tile_pool(name="qkv", bufs=2, space="PSUM") as qkv_pool,
        ):
            # ---- persistent SBUF tensors ----
            xt_sb = singles.tile([P, DC, T], bf16, tag="xt")
            wq_sb = singles.tile([P, DC, F], bf16, tag="wq")
            wk_sb = singles.tile([P, DC, F], bf16, tag="wk")
            wv_sb = singles.tile([P, DC, F], bf16, tag="wv")
            bq_sb = singles.tile([P, NPAIR], fp32, tag="bq")
            bk_sb = singles.tile([P, NPAIR], fp32, tag="bk")
            bv_sb = singles.tile([P, F], fp32, tag="bv")
            # per-pair Q^T/K^T [feat-in-pair, t] and V [t-in-ktile, kt, hp, 65]
            qt_sb = [singles.tile([P, T], bf16, tag=f"qt{j}", name=f"qt{j}")
                     for j in range(NPAIR)]
            kt_sb = [singles.tile([P, T], bf16, tag=f"kt{j}", name=f"kt{j}")
                     for j in range(NPAIR)]
            v_sb = [singles.tile([P, NKT, 2, HD + 1], bf16, tag=f"v{j}",
                                 name=f"v{j}")
                    for j in range(NPAIR)]
            # normalize staging, separate per head-slot (a/b). The [1, 512]
            # Z row would use one DVE lane (3.3us reciprocal), so bounce it
            # through a [128, 4] layout via sb->sb DMA: reciprocal runs on
            # 128 lanes, and the gather-back lands on partition 0 (the only
            # partition gpsimd's partition_broadcast can read on HW).
            zcol = [singles.tile([P, 4], fp32, tag=f"zcol{i}",
                                 name=f"zcol{i}") for i in range(2)]
            rz0 = [singles.tile([1, QC], fp32, tag=f"rz0{i}",
                                name=f"rz0{i}") for i in range(2)]
            rzb = [singles.tile([HD, QC], fp32, tag=f"rzb{i}",
                                name=f"rzb{i}") for i in range(2)]

            nc.sync.dma_start(out=xt_sb[:], in_=xt[:])
            nc.sync.dma_start(out=wq_sb[:], in_=wq[:])
            nc.sync.dma_start(out=wk_sb[:], in_=wk[:])
            nc.sync.dma_start(out=wv_sb[:], in_=wv[:])
            nc.sync.dma_start(out=bq_sb[:], in_=bq[:])
            nc.sync.dma_start(out=bk_sb[:], in_=bk[:])
            nc.sync.dma_start(out=bv_sb[:], in_=bv[:])
            for j in range(NPAIR):
                nc.vector.memset(v_sb[j][:, :, :, HD:HD + 1], 1.0)

            def emit_qk_chunk(j, which, tcn):
                """One [f=128, t=512] t-chunk of Q^T or K^T for pair j."""
                w_sb, b_sb, dst = ((wq_sb, bq_sb, qt_sb[j]) if which == "q"
                                   else (wk_sb, bk_sb, kt_sb[j]))
                ps = qkv_pool.tile([P, 512], fp32, tag="qkv", name="qkps")
                for dc in range(DC):
                    nc.tensor.matmul(
                        ps[:],
                        w_sb[:, dc, P * j:P * (j + 1)],
                        xt_sb[:, dc, 512 * tcn:512 * (tcn + 1)],
                        start=(dc == 0), stop=(dc == DC - 1),
                    )
                nc.vector.tensor_scalar_add(
                    out=dst[:, 512 * tcn:512 * (tcn + 1)],
                    in0=ps[:],
                    scalar1=b_sb[:, j:j + 1],
                )

            def emit_qk_proj(j):
                for which in ("q", "k"):
                    for tcn in range(T // 512):
                        emit_qk_chunk(j, which, tcn)

            def emit_v_proj(tt_lo, tt_hi):
                """V rows, all pairs at once: psum [t=128, f=512] per t-tile."""
                for tt in range(tt_lo, tt_hi):
                    ps = qkv_pool.tile([P, F], fp32, tag="qkv")
                    for dc in range(DC):
                        nc.tensor.matmul(
                            ps[:],
                            xt_sb[:, dc, P * tt:P * (tt + 1)],
                            wv_sb[:, dc, :],
                            start=(dc == 0), stop=(dc == DC - 1),
                        )
                    for j in range(NPAIR):
                        nc.vector.tensor_add(
                            out=v_sb[j][:, tt, :, 0:HD],
                            in0=ps[:, P * j:P * (j + 1)].rearrange(
                                "p (h d) -> p h d", h=2),
                            in1=bv_sb[:, P * j:P * (j + 1)].rearrange(
                                "p (h d) -> p h d", h=2),
                        )

            NTC = T // 512
            # prologue: all of K^T(0) + first chunk of Q^T(0). V is NOT in
            # the prologue: the first iteration's scores/exp only need Q/K,
            # so the scalar engine (the saturated engine) starts ~30us
            # earlier and the V matmuls overlap with the first exps; only
            # that iteration's PV waits for V.
            for tcn in range(NTC):
                emit_qk_chunk(0, "k", tcn)
            emit_qk_chunk(0, "q", 0)

            def emit_scores_exp(j, qc, ktn):
                qt, kt = qt_sb[j], kt_sb[j]
                q0 = QC * qc
                # scores S^T[k, q] for BOTH heads of the pair in one
                # 2-bank psum tile: head A on PE rows 0-63, head B
                # on rows 64-127. Sharing one tile makes the two
                # matmuls ready simultaneously, so the scheduler
                # keeps them adjacent and the row-disjoint matmuls
                # run concurrently on the array (~2x).
                s = sps_pool.tile([P, 2, QC], fp32, tag="sps", name="s")
                for hp in (0, 1):
                    nc.tensor.matmul(
                        s[:, hp, :],
                        kt[HD * hp:HD * (hp + 1), P * ktn:P * (ktn + 1)],
                        qt[HD * hp:HD * (hp + 1), q0:q0 + QC],
                        start=True, stop=True,
                    )
                es = es_pool.tile([P, 2, QC], bf16, tag="es", name="es")
                nc.scalar.activation(
                    es[:].rearrange("p a b -> p (a b)"),
                    s[:].rearrange("p a b -> p (a b)"),
                    Exp, scale=0.125)
                return es

            def emit_pv(j, qc, ktn, es, pva, pvb):
                vv = v_sb[j]
                first = ktn == 0
                last = ktn == NKT - 1
                nc.tensor.matmul(pva[:], vv[:, ktn, 0, :], es[:, 0, :],
                                 start=first, stop=last)
                nc.tensor.matmul(pvb[:], vv[:, ktn, 1, :], es[:, 1, :],
                                 start=first, stop=last)

            for j in range(NPAIR):
                for qc in range(NQC):
                    q0 = QC * qc
                    pva = pv_pool.tile([HD + 1, QC], fp32, tag="pva")
                    pvb = pv_pool.tile([HD + 1, QC], fp32, tag="pvb")
                    if j == 0 and qc == 0:
                        # first iteration: all scores/exp first, then the V
                        # projection, then the deferred PV accumulation
                        es_list = [emit_scores_exp(j, qc, ktn)
                                   for ktn in range(NKT)]
                        emit_v_proj(0, NKT)
                        for ktn, es in enumerate(es_list):
                            emit_pv(j, qc, ktn, es, pva, pvb)
                    else:
                        for ktn in range(NKT):
                            # keep the PE warm through ACT-paced stretches:
                            # the last pair has no projection filler left,
                            # so issue tiny throwaway matmuls (HAM
                            # re-throttles the PE clock after ~3.4us of
                            # contiguous idle)
                            if j == NPAIR - 1 and ktn % 2 == 0:
                                dm = qkv_pool.tile([P, 256], fp32,
                                                   tag="qkv", name="warmmm")
                                nc.tensor.matmul(
                                    dm[:], wq_sb[:, 0, 0:P],
                                    xt_sb[:, 0, 0:256],
                                    start=True, stop=True)
                            es = emit_scores_exp(j, qc, ktn)
                            emit_pv(j, qc, ktn, es, pva, pvb)
                    # normalize: row HD of pv holds Z = sum_k exp(s/8).
                    # Copy psum->sbuf first so the PV banks free up fast
                    # (the recip/broadcast chain is slow but off-critical).
                    pvcs = []
                    for hp, pv_t in ((0, pva), (1, pvb)):
                        pvc = norm_pool.tile([HD + 1, QC], fp32,
                                             tag=f"pvc{hp}", name=f"pvc{hp}")
                        nc.vector.tensor_copy(pvc[:], pv_t[:])
                        pvcs.append(pvc)
                    for hp in (0, 1):
                        pvc = pvcs[hp]
                        nc.sync.dma_start(out=zcol[hp][:],
                                          in_=pvc[HD:HD + 1, :])
                        nc.vector.reciprocal(zcol[hp][:], zcol[hp][:])
                        nc.sync.dma_start(out=rz0[hp][:], in_=zcol[hp][:])
                        nc.gpsimd.partition_broadcast(rzb[hp][:], rz0[hp][:])
                        st = stage_pool.tile([HD, QC], fp32, tag=f"st{hp}",
                                             name=f"st{hp}")
                        nc.vector.tensor_mul(st[:], pvc[0:HD, :],
                                             rzb[hp][:])
                        nc.sync.dma_start(out=o[2 * j + hp, :, q0:q0 + QC],
                                          in_=st[:])
                    # feed the PE pipeline with projection filler, spread
                    # over every iteration: next pair's K^T chunk-by-chunk,
                    # this pair's remaining Q^T chunks just before use, and
                    # next pair's first Q^T chunk at the boundary
                    for tcn in range(qc * NTC // NQC, (qc + 1) * NTC // NQC):
                        if j + 1 < NPAIR:
                            emit_qk_chunk(j + 1, "k", tcn)
                    nxt = (qc + 1) * NTC // NQC
                    if nxt < NTC:
                        emit_qk_chunk(j, "q", nxt)
                    elif j + 1 < NPAIR:
                        emit_qk_chunk(j + 1, "q", 0)

    nc.compile()
    return nc


def _prep_inputs(x, Wq, bq, Wk, bk, Wv, bv):
    """Host-side shard + layout prep. Returns per-core input dicts."""
    in_maps = []
    xt_cache = {}
    w_cache = {}
    for c in range(N_CORES):
        b, g = c // G, c % G
        if b not in xt_cache:
            xtb = np.ascontiguousarray(x[b].T).astype(BF16)      # [D, T]
            xt_cache[b] = np.ascontiguousarray(
                xtb.reshape(DC, P, T).transpose(1, 0, 2))        # [P, DC, T]
        if g not in w_cache:
            def _w(W):
                Wg = W[:, F * g:F * (g + 1)].astype(BF16)        # [D, F]
                return np.ascontiguousarray(
                    Wg.reshape(DC, P, F).transpose(1, 0, 2))     # [P, DC, F]
            bqg = bq[F * g:F * (g + 1)].astype(np.float32)
            bkg = bk[F * g:F * (g + 1)].astype(np.float32)
            bvg = bv[F * g:F * (g + 1)].astype(np.float32)
            w_cache[g] = {
                "wq": _w(Wq), "wk": _w(Wk), "wv": _w(Wv),
                # [P, NPAIR]: bias for feature 128*j + p
                "bq": np.ascontiguousarray(bqg.reshape(NPAIR, P).T),
                "bk": np.ascontiguousarray(bkg.reshape(NPAIR, P).T),
                # [P, F]: broadcast along partitions
                "bv": np.ascontiguousarray(
                    np.broadcast_to(bvg[None, :], (P, F))),
            }
        in_maps.append({"xt": xt_cache[b], **w_cache[g]})
    return in_maps


def _run(in_maps, trace_dir=None, trace_cores=None):
    from concourse.bass_utils import run_bass_kernel_spmd

    global _compiled
    if _compiled is None:
        _compiled = _build()
    nc = _compiled

    if trace_dir is not None:
        from trn_agent_boot.trn_boot import _ntff_profile_via_ctypes
        hook = _ntff_profile_via_ctypes("/opt/axon/libaxon_pjrt.so")
        with hook(trace_dir, trace_cores):
            res = run_bass_kernel_spmd(nc, in_maps,
                                       core_ids=list(range(N_CORES)))
    else:
        res = run_bass_kernel_spmd(nc, in_maps, core_ids=list(range(N_CORES)))
    return res


def kernel(x, Wq, bq, Wk, bk, Wv, bv, _trace_dir=None, _trace_cores=None):
    x = np.asarray(x, dtype=np.float32)
    in_maps = _prep_inputs(x, np.asarray(Wq), np.asarray(bq), np.asarray(Wk),
                           np.asarray(bk), np.asarray(Wv), np.asarray(bv))
    res = _run(in_maps, _trace_dir, _trace_cores)
    out = np.empty((B, T, D), np.float32)
    for c in range(N_CORES):
        b, g = c // G, c % G
        oc = np.asarray(res.results[c]["o"])          # [HPC, HD, T]
        out[b, :, F * g:F * (g + 1)] = (
            oc.transpose(2, 0, 1).reshape(T, F))
    return out


# revision 28
# speedup vs baseline: 1.7629x; 1.0136x over previous
"""Multi-head attention kernel for Trainium2, 8 NeuronCores.

Problem: B=4, T=2048, D=1024, H=16 heads, head_dim=64.
Sharding: core c -> batch b = c//2, head group g = c%2 (8 heads each).
Each core computes QKV projections for its 512 features and full
attention for its 8 heads over its batch. No cross-core communication.

Per-core layout (all matmul inputs bf16, fp32 accumulation):
  - x is passed transposed+chunked: xt[p, dc, t] = x[b, t, 128*dc+p]
  - weights passed chunked:  wq[p, dc, f] = Wq[128*dc+p, 512*g+f]
  - Q^T/K^T computed feature-major [feat, t] so attention scores
    S^T[k, q] = sum_d K^T[d, k] Q^T[d, q] come out with k on partitions
  - V computed in natural [t, f] layout, augmented with a ones column:
    PV matmul accumulates [65, 512] where row 64 = softmax denominator
  - softmax needs no max subtraction: |S/8| <= ~7 for N(0,1) inputs
  - output written per head as O^T [64, t]; host transposes/concats
"""

import os
import sys

for _p in ("/opt/trn_rl_repo", "/opt/pypackages"):
    if _p not in sys.path:
        sys.path.insert(0, _p)

import numpy as np
import ml_dtypes

B, T, D, H = 4, 2048, 1024, 16
HD = D // H            # 64 head dim
N_CORES = 8
G = 2                  # head groups (cores per batch)
F = D // G             # 512 features per core
HPC = H // G           # 8 heads per core
P = 128
DC = D // P            # 8 contraction chunks
NPAIR = HPC // 2       # 4 head pairs per core
QC = 512               # query-chunk (columns per score matmul)
NQC = T // QC          # 4 query chunks
NKT = T // P           # 16 key tiles

BF16 = ml_dtypes.bfloat16

_compiled = None  # (nc,) cached across calls in one process


def _build():
    import concourse.bass as bass
    import concourse.tile as tile
    from concourse import bacc, mybir

    fp32 = mybir.dt.float32
    bf16 = mybir.dt.bfloat16
    Exp = mybir.ActivationFunctionType.Exp

    nc = bacc.Bacc("TRN2", target_bir_lowering=False, debug=False,
                   num_devices=N_CORES)

    xt = nc.dram_tensor("xt", [P, DC, T], bf16, kind="ExternalInput").ap()
    wq = nc.dram_tensor("wq", [P, DC, F], bf16, kind="ExternalInput").ap()
    wk = nc.dram_tensor("wk", [P, DC, F], bf16, kind="ExternalInput").ap()
    wv = nc.dram_tensor("wv", [P, DC, F], bf16, kind="ExternalInput").ap()
    bq = nc.dram_tensor("bq", [P, NPAIR], fp32, kind="ExternalInput").ap()
    bk = nc.dram_tensor("bk", [P, NPAIR], fp32, kind="ExternalInput").ap()
    bv = nc.dram_tensor("bv", [P, F], fp32, kind="ExternalInput").ap()
    o = nc.dram_tensor("o", [HPC, HD, T], fp32, kind="ExternalOutput").ap()

    with tile.TileContext(nc) as tc:
        with (
            tc.tile_pool(name="singles", bufs=1) as singles,
            tc.tile_pool(name="es", bufs=6) as es_pool,
            tc.tile_pool(name="stage", bufs=2) as stage_pool,
            tc.tile_pool(name="norm", bufs=2) as norm_pool,
            tc.tile_pool(name="sps", bufs=2, space="PSUM") as sps_pool,
            tc.tile_pool(name="pv", bufs=1, space="PSUM") as pv_pool,
            tc.tile_pool(name="qkv", bufs=2, space="PSUM") as qkv_pool,
        ):
            # ---- persistent SBUF tensors ----
            xt_sb = singles.tile([P, DC, T], bf16, tag="xt")
            wq_sb = singles.tile([P, DC, F], bf16, tag="wq")
            wk_sb = singles.tile([P, DC, F], bf16, tag="wk")
            wv_sb = singles.tile([P, DC, F], bf16, tag="wv")
            bq_sb = singles.tile([P, NPAIR], fp32, tag="bq")
            bk_sb = singles.tile([P, NPAIR], fp32, tag="bk")
            bv_sb = singles.tile([P, F], fp32, tag="bv")
            # per-pair Q^T/K^T [feat-in-pair, t] and V [t-in-ktile, kt, hp, 65]
            qt_sb = [singles.tile([P, T], bf16, tag=f"qt{j}", name=f"qt{j}")
                     for j in range(NPAIR)]
            kt_sb = [singles.tile([P, T], bf16, tag=f"kt{j}", name=f"kt{j}")
                     for j in range(NPAIR)]
            v_sb = [singles.tile([P, NKT, 2, HD + 1], bf16, tag=f"v{j}",
                                 name=f"v{j}")
                    for j in range(NPAIR)]
            # normalize staging, separate per head-slot (a/b). The [1, 512]
            # Z row would use one DVE lane (3.3us reciprocal), so bounce it
            # through a [128, 4] layout via sb->sb DMA: reciprocal runs on
            # 128 lanes, and the gather-back lands on partition 0 (the only
            # partition gpsimd's partition_broadcast can read on HW).
            zcol = [singles.tile([P, 4], fp32, tag=f"zcol{i}",
                                 name=f"zcol{i}") for i in range(2)]
            rz0 = [singles.tile([1, QC], fp32, tag=f"rz0{i}",
                                name=f"rz0{i}") for i in range(2)]
            rzb = [singles.tile([HD, QC], fp32, tag=f"rzb{i}",
                                name=f"rzb{i}") for i in range(2)]

            # load order matters for startup latency: the first projection
            # chunks need wk + the first xt t-columns, so land those first
            nc.sync.dma_start(out=wk_sb[:], in_=wk[:])
            nc.sync.dma_start(out=bk_sb[:], in_=bk[:])
            for tcn in range(4):
                nc.sync.dma_start(out=xt_sb[:, :, 512 * tcn:512 * (tcn + 1)],
                                  in_=xt[:, :, 512 * tcn:512 * (tcn + 1)])
            nc.sync.dma_start(out=wq_sb[:], in_=wq[:])
            nc.sync.dma_start(out=bq_sb[:], in_=bq[:])
            nc.sync.dma_start(out=wv_sb[:], in_=wv[:])
            nc.sync.dma_start(out=bv_sb[:], in_=bv[:])
            for j in range(NPAIR):
                nc.vector.memset(v_sb[j][:, :, :, HD:HD + 1], 1.0)

            def emit_qk_chunk(j, which, tcn):
                """One [f=128, t=512] t-chunk of Q^T or K^T for pair j."""
                w_sb, b_sb, dst = ((wq_sb, bq_sb, qt_sb[j]) if which == "q"
                                   else (wk_sb, bk_sb, kt_sb[j]))
                ps = qkv_pool.tile([P, 512], fp32, tag="qkv", name="qkps")
                for dc in range(DC):
                    nc.tensor.matmul(
                        ps[:],
                        w_sb[:, dc, P * j:P * (j + 1)],
                        xt_sb[:, dc, 512 * tcn:512 * (tcn + 1)],
                        start=(dc == 0), stop=(dc == DC - 1),
                    )
                nc.vector.tensor_scalar_add(
                    out=dst[:, 512 * tcn:512 * (tcn + 1)],
                    in0=ps[:],
                    scalar1=b_sb[:, j:j + 1],
                )

            def emit_qk_proj(j):
                for which in ("q", "k"):
                    for tcn in range(T // 512):
                        emit_qk_chunk(j, which, tcn)

            def emit_v_proj(tt_lo, tt_hi):
                """V rows, all pairs at once: psum [t=128, f=512] per t-tile."""
                for tt in range(tt_lo, tt_hi):
                    ps = qkv_pool.tile([P, F], fp32, tag="qkv")
                    for dc in range(DC):
                        nc.tensor.matmul(
                            ps[:],
                            xt_sb[:, dc, P * tt:P * (tt + 1)],
                            wv_sb[:, dc, :],
                            start=(dc == 0), stop=(dc == DC - 1),
                        )
                    for j in range(NPAIR):
                        nc.vector.tensor_add(
                            out=v_sb[j][:, tt, :, 0:HD],
                            in0=ps[:, P * j:P * (j + 1)].rearrange(
                                "p (h d) -> p h d", h=2),
                            in1=bv_sb[:, P * j:P * (j + 1)].rearrange(
                                "p (h d) -> p h d", h=2),
                        )

            NTC = T // 512
            # prologue: all of K^T(0) + first chunk of Q^T(0). V is NOT in
            # the prologue: the first iteration's scores/exp only need Q/K,
            # so the scalar engine (the saturated engine) starts ~30us
            # earlier and the V matmuls overlap with the first exps; only
            # that iteration's PV waits for V.
            for tcn in range(NTC):
                emit_qk_chunk(0, "k", tcn)
            emit_qk_chunk(0, "q", 0)

            def emit_scores_exp(j, qc, ktn):
                qt, kt = qt_sb[j], kt_sb[j]
                q0 = QC * qc
                # scores S^T[k, q] for BOTH heads of the pair in one
                # 2-bank psum tile: head A on PE rows 0-63, head B
                # on rows 64-127. Sharing one tile makes the two
                # matmuls ready simultaneously, so the scheduler
                # keeps them adjacent and the row-disjoint matmuls
                # run concurrently on the array (~2x).
                s = sps_pool.tile([P, 2, QC], fp32, tag="sps", name="s")
                for hp in (0, 1):
                    nc.tensor.matmul(
                        s[:, hp, :],
                        kt[HD * hp:HD * (hp + 1), P * ktn:P * (ktn + 1)],
                        qt[HD * hp:HD * (hp + 1), q0:q0 + QC],
                        start=True, stop=True,
                    )
                es = es_pool.tile([P, 2, QC], bf16, tag="es", name="es")
                nc.scalar.activation(
                    es[:].rearrange("p a b -> p (a b)"),
                    s[:].rearrange("p a b -> p (a b)"),
                    Exp, scale=0.125)
                return es

            def emit_pv(j, qc, ktn, es, pva, pvb):
                vv = v_sb[j]
                first = ktn == 0
                last = ktn == NKT - 1
                nc.tensor.matmul(pva[:], vv[:, ktn, 0, :], es[:, 0, :],
                                 start=first, stop=last)
                nc.tensor.matmul(pvb[:], vv[:, ktn, 1, :], es[:, 1, :],
                                 start=first, stop=last)

            for j in range(NPAIR):
                for qc in range(NQC):
                    q0 = QC * qc
                    pva = pv_pool.tile([HD + 1, QC], fp32, tag="pva")
                    pvb = pv_pool.tile([HD + 1, QC], fp32, tag="pvb")
                    if j == 0 and qc == 0:
                        # first iteration: pipeline the V projection with
                        # the attention — PV for k-tile kt only needs V
                        # t-tile kt, so V tiles are produced just-in-time
                        # while the scalar engine works through the exps
                        for ktn in range(NKT):
                            es = emit_scores_exp(j, qc, ktn)
                            emit_v_proj(ktn, ktn + 1)
                            emit_pv(j, qc, ktn, es, pva, pvb)
                    else:
                        for ktn in range(NKT):
                            # keep the PE warm through ACT-paced stretches:
                            # the last pair has no projection filler left,
                            # so issue tiny throwaway matmuls (HAM
                            # re-throttles the PE clock after ~3.4us of
                            # contiguous idle)
                            if j == NPAIR - 1 and ktn % 2 == 0:
                                dm = qkv_pool.tile([P, 256], fp32,
                                                   tag="qkv", name="warmmm")
                                nc.tensor.matmul(
                                    dm[:], wq_sb[:, 0, 0:P],
                                    xt_sb[:, 0, 0:256],
                                    start=True, stop=True)
                            es = emit_scores_exp(j, qc, ktn)
                            emit_pv(j, qc, ktn, es, pva, pvb)
                    # normalize: row HD of pv holds Z = sum_k exp(s/8).
                    # Copy psum->sbuf first so the PV banks free up fast
                    # (the recip/broadcast chain is slow but off-critical).
                    pvcs = []
                    for hp, pv_t in ((0, pva), (1, pvb)):
                        pvc = norm_pool.tile([HD + 1, QC], fp32,
                                             tag=f"pvc{hp}", name=f"pvc{hp}")
                        nc.vector.tensor_copy(pvc[:], pv_t[:])
                        pvcs.append(pvc)
                    for hp in (0, 1):
                        pvc = pvcs[hp]
                        nc.sync.dma_start(out=zcol[hp][:],
                                          in_=pvc[HD:HD + 1, :])
                        nc.vector.reciprocal(zcol[hp][:], zcol[hp][:])
                        nc.sync.dma_start(out=rz0[hp][:], in_=zcol[hp][:])
                        nc.gpsimd.partition_broadcast(rzb[hp][:], rz0[hp][:])
                        st = stage_pool.tile([HD, QC], fp32, tag=f"st{hp}",
                                             name=f"st{hp}")
                        nc.vector.tensor_mul(st[:], pvc[0:HD, :],
                                             rzb[hp][:])
                        nc.sync.dma_start(out=o[2 * j + hp, :, q0:q0 + QC],
                                          in_=st[:])
                    # feed the PE pipeline with projection filler, spread
                    # over every iteration: next pair's K^T chunk-by-chunk,
                    # this pair's remaining Q^T chunks just before use, and
                    # next pair's first Q^T chunk at the boundary
                    for tcn in range(qc * NTC // NQC, (qc + 1) * NTC // NQC):
                        if j + 1 < NPAIR:
                            emit_qk_chunk(j + 1, "k", tcn)
                    nxt = (qc + 1) * NTC // NQC
                    if nxt < NTC:
                        emit_qk_chunk(j, "q", nxt)
                    elif j + 1 < NPAIR:
                        emit_qk_chunk(j + 1, "q", 0)

    nc.compile()
    return nc


def _prep_inputs(x, Wq, bq, Wk, bk, Wv, bv):
    """Host-side shard + layout prep. Returns per-core input dicts."""
    in_maps = []
    xt_cache = {}
    w_cache = {}
    for c in range(N_CORES):
        b, g = c // G, c % G
        if b not in xt_cache:
            xtb = np.ascontiguousarray(x[b].T).astype(BF16)      # [D, T]
            xt_cache[b] = np.ascontiguousarray(
                xtb.reshape(DC, P, T).transpose(1, 0, 2))        # [P, DC, T]
        if g not in w_cache:
            def _w(W):
                Wg = W[:, F * g:F * (g + 1)].astype(BF16)        # [D, F]
                return np.ascontiguousarray(
                    Wg.reshape(DC, P, F).transpose(1, 0, 2))     # [P, DC, F]
            bqg = bq[F * g:F * (g + 1)].astype(np.float32)
            bkg = bk[F * g:F * (g + 1)].astype(np.float32)
            bvg = bv[F * g:F * (g + 1)].astype(np.float32)
            w_cache[g] = {
                "wq": _w(Wq), "wk": _w(Wk), "wv": _w(Wv),
                # [P, NPAIR]: bias for feature 128*j + p
                "bq": np.ascontiguousarray(bqg.reshape(NPAIR, P).T),
                "bk": np.ascontiguousarray(bkg.reshape(NPAIR, P).T),
                # [P, F]: broadcast along partitions
                "bv": np.ascontiguousarray(
                    np.broadcast_to(bvg[None, :], (P, F))),
            }
        in_maps.append({"xt": xt_cache[b], **w_cache[g]})
    return in_maps


def _run(in_maps, trace_dir=None, trace_cores=None):
    from concourse.bass_utils import run_bass_kernel_spmd

    global _compiled
    if _compiled is None:
        _compiled = _build()
    nc = _compiled

    if trace_dir is not None:
        from trn_agent_boot.trn_boot import _ntff_profile_via_ctypes
        hook = _ntff_profile_via_ctypes("/opt/axon/libaxon_pjrt.so")
        with hook(trace_dir, trace_cores):
            res = run_bass_kernel_spmd(nc, in_maps,
                                       core_ids=list(range(N_CORES)))
    else:
        res = run_bass_kernel_spmd(nc, in_maps, core_ids=list(range(N_CORES)))
    return res


def kernel(x, Wq, bq, Wk, bk, Wv, bv, _trace_dir=None, _trace_cores=None):
    x = np.asarray(x, dtype=np.float32)
    in_maps = _prep_inputs(x, np.asarray(Wq), np.asarray(bq), np.asarray(Wk),
                           np.asarray(bk), np.asarray(Wv), np.asarray(bv))
    res = _run(in_maps, _trace_dir, _trace_cores)
    out = np.empty((B, T, D), np.float32)
    for c in range(N_CORES):
        b, g = c // G, c % G
        oc = np.asarray(res.results[c]["o"])          # [HPC, HD, T]
        out[b, :, F * g:F * (g + 1)] = (
            oc.transpose(2, 0, 1).reshape(T, F))
    return out


# revision 31
# speedup vs baseline: 1.7804x; 1.0099x over previous
"""Multi-head attention kernel for Trainium2, 8 NeuronCores.

Problem: B=4, T=2048, D=1024, H=16 heads, head_dim=64.
Sharding: core c -> batch b = c//2, head group g = c%2 (8 heads each).
Each core computes QKV projections for its 512 features and full
attention for its 8 heads over its batch. No cross-core communication.

Per-core layout (all matmul inputs bf16, fp32 accumulation):
  - x is passed transposed+chunked: xt[p, dc, t] = x[b, t, 128*dc+p]
  - weights passed chunked:  wq[p, dc, f] = Wq[128*dc+p, 512*g+f]
  - Q^T/K^T computed feature-major [feat, t] so attention scores
    S^T[k, q] = sum_d K^T[d, k] Q^T[d, q] come out with k on partitions
  - V computed in natural [t, f] layout, augmented with a ones column:
    PV matmul accumulates [65, 512] where row 64 = softmax denominator
  - softmax needs no max subtraction: |S/8| <= ~7 for N(0,1) inputs
  - output written per head as O^T [64, t]; host transposes/concats
"""

import os
import sys

for _p in ("/opt/trn_rl_repo", "/opt/pypackages"):
    if _p not in sys.path:
        sys.path.insert(0, _p)

import numpy as np
import ml_dtypes

B, T, D, H = 4, 2048, 1024, 16
HD = D // H            # 64 head dim
N_CORES = 8
G = 2                  # head groups (cores per batch)
F = D // G             # 512 features per core
HPC = H // G           # 8 heads per core
P = 128
DC = D // P            # 8 contraction chunks
NPAIR = HPC // 2       # 4 head pairs per core
QC = 512               # query-chunk (columns per score matmul)
NQC = T // QC          # 4 query chunks
NKT = T // P           # 16 key tiles

BF16 = ml_dtypes.bfloat16

_compiled = None  # (nc,) cached across calls in one process


def _build():
    import concourse.bass as bass
    import concourse.tile as tile
    from concourse import bacc, mybir

    fp32 = mybir.dt.float32
    bf16 = mybir.dt.bfloat16
    Exp = mybir.ActivationFunctionType.Exp

    nc = bacc.Bacc("TRN2", target_bir_lowering=False, debug=False,
                   num_devices=N_CORES)

    xt = nc.dram_tensor("xt", [P, DC, T], bf16, kind="ExternalInput").ap()
    wq = nc.dram_tensor("wq", [P, DC, F], bf16, kind="ExternalInput").ap()
    wk = nc.dram_tensor("wk", [P, DC, F], bf16, kind="ExternalInput").ap()
    wv = nc.dram_tensor("wv", [P, DC, F], bf16, kind="ExternalInput").ap()
    bq = nc.dram_tensor("bq", [P, NPAIR], fp32, kind="ExternalInput").ap()
    bk = nc.dram_tensor("bk", [P, NPAIR], fp32, kind="ExternalInput").ap()
    bv = nc.dram_tensor("bv", [P, F], fp32, kind="ExternalInput").ap()
    o = nc.dram_tensor("o", [HPC, HD, T], fp32, kind="ExternalOutput").ap()

    with tile.TileContext(nc) as tc:
        with (
            tc.tile_pool(name="singles", bufs=1) as singles,
            tc.tile_pool(name="es", bufs=6) as es_pool,
            tc.tile_pool(name="stage", bufs=2) as stage_pool,
            tc.tile_pool(name="norm", bufs=2) as norm_pool,
            tc.tile_pool(name="sps", bufs=2, space="PSUM") as sps_pool,
            tc.tile_pool(name="pv", bufs=1, space="PSUM") as pv_pool,
            tc.tile_pool(name="qkv", bufs=2, space="PSUM") as qkv_pool,
        ):
            # ---- persistent SBUF tensors ----
            xt_sb = singles.tile([P, DC, T], bf16, tag="xt")
            wq_sb = singles.tile([P, DC, F], bf16, tag="wq")
            wk_sb = singles.tile([P, DC, F], bf16, tag="wk")
            wv_sb = singles.tile([P, DC, F], bf16, tag="wv")
            bq_sb = singles.tile([P, NPAIR], fp32, tag="bq")
            bk_sb = singles.tile([P, NPAIR], fp32, tag="bk")
            bv_sb = singles.tile([P, F], fp32, tag="bv")
            # per-pair Q^T/K^T [feat-in-pair, t] and V [t-in-ktile, kt, hp, 65]
            qt_sb = [singles.tile([P, T], bf16, tag=f"qt{j}", name=f"qt{j}")
                     for j in range(NPAIR)]
            kt_sb = [singles.tile([P, T], bf16, tag=f"kt{j}", name=f"kt{j}")
                     for j in range(NPAIR)]
            v_sb = [singles.tile([P, NKT, 2, HD + 1], bf16, tag=f"v{j}",
                                 name=f"v{j}")
                    for j in range(NPAIR)]
            # normalize staging, separate per head-slot (a/b). The [1, 512]
            # Z row would use one DVE lane (3.3us reciprocal), so bounce it
            # through a [128, 4] layout via sb->sb DMA: reciprocal runs on
            # 128 lanes, and the gather-back lands on partition 0 (the only
            # partition gpsimd's partition_broadcast can read on HW).
            zcol = [singles.tile([P, 4], fp32, tag=f"zcol{i}",
                                 name=f"zcol{i}") for i in range(2)]
            rz0 = [singles.tile([1, QC], fp32, tag=f"rz0{i}",
                                name=f"rz0{i}") for i in range(2)]
            rzb = [singles.tile([HD, QC], fp32, tag=f"rzb{i}",
                                name=f"rzb{i}") for i in range(2)]

            # load order matters for startup latency: the first projection
            # chunks need wk + the first xt t-columns, so land those first
            nc.sync.dma_start(out=wk_sb[:], in_=wk[:])
            nc.sync.dma_start(out=bk_sb[:], in_=bk[:])
            nc.sync.dma_start(out=xt_sb[:, :, 0:512], in_=xt[:, :, 0:512])
            nc.sync.dma_start(out=wq_sb[:], in_=wq[:])
            nc.sync.dma_start(out=bq_sb[:], in_=bq[:])
            for tcn in range(1, T // 512):
                nc.sync.dma_start(out=xt_sb[:, :, 512 * tcn:512 * (tcn + 1)],
                                  in_=xt[:, :, 512 * tcn:512 * (tcn + 1)])
            nc.sync.dma_start(out=wv_sb[:], in_=wv[:])
            nc.sync.dma_start(out=bv_sb[:], in_=bv[:])
            for j in range(NPAIR):
                nc.vector.memset(v_sb[j][:, :, :, HD:HD + 1], 1.0)

            def emit_qk_chunk(j, which, tcn):
                """One [f=128, t=512] t-chunk of Q^T or K^T for pair j."""
                w_sb, b_sb, dst = ((wq_sb, bq_sb, qt_sb[j]) if which == "q"
                                   else (wk_sb, bk_sb, kt_sb[j]))
                ps = qkv_pool.tile([P, 512], fp32, tag="qkv", name="qkps")
                for dc in range(DC):
                    nc.tensor.matmul(
                        ps[:],
                        w_sb[:, dc, P * j:P * (j + 1)],
                        xt_sb[:, dc, 512 * tcn:512 * (tcn + 1)],
                        start=(dc == 0), stop=(dc == DC - 1),
                    )
                nc.vector.tensor_scalar_add(
                    out=dst[:, 512 * tcn:512 * (tcn + 1)],
                    in0=ps[:],
                    scalar1=b_sb[:, j:j + 1],
                )

            def emit_qk_proj(j):
                for which in ("q", "k"):
                    for tcn in range(T // 512):
                        emit_qk_chunk(j, which, tcn)

            def emit_v_proj(tt_lo, tt_hi):
                """V rows, all pairs at once: psum [t=128, f=512] per t-tile."""
                for tt in range(tt_lo, tt_hi):
                    ps = qkv_pool.tile([P, F], fp32, tag="qkv")
                    for dc in range(DC):
                        nc.tensor.matmul(
                            ps[:],
                            xt_sb[:, dc, P * tt:P * (tt + 1)],
                            wv_sb[:, dc, :],
                            start=(dc == 0), stop=(dc == DC - 1),
                        )
                    for j in range(NPAIR):
                        nc.vector.tensor_add(
                            out=v_sb[j][:, tt, :, 0:HD],
                            in0=ps[:, P * j:P * (j + 1)].rearrange(
                                "p (h d) -> p h d", h=2),
                            in1=bv_sb[:, P * j:P * (j + 1)].rearrange(
                                "p (h d) -> p h d", h=2),
                        )

            NTC = T // 512
            # prologue: all of K^T(0) + first chunk of Q^T(0). V is NOT in
            # the prologue: the first iteration's scores/exp only need Q/K,
            # so the scalar engine (the saturated engine) starts ~30us
            # earlier and the V matmuls overlap with the first exps; only
            # that iteration's PV waits for V.
            emit_qk_chunk(0, "k", 0)
            emit_qk_chunk(0, "q", 0)
            for tcn in range(1, NTC):
                emit_qk_chunk(0, "k", tcn)

            def emit_scores_exp(j, qc, ktn):
                qt, kt = qt_sb[j], kt_sb[j]
                q0 = QC * qc
                # scores S^T[k, q] for BOTH heads of the pair in one
                # 2-bank psum tile: head A on PE rows 0-63, head B
                # on rows 64-127. Sharing one tile makes the two
                # matmuls ready simultaneously, so the scheduler
                # keeps them adjacent and the row-disjoint matmuls
                # run concurrently on the array (~2x).
                s = sps_pool.tile([P, 2, QC], fp32, tag="sps", name="s")
                for hp in (0, 1):
                    nc.tensor.matmul(
                        s[:, hp, :],
                        kt[HD * hp:HD * (hp + 1), P * ktn:P * (ktn + 1)],
                        qt[HD * hp:HD * (hp + 1), q0:q0 + QC],
                        start=True, stop=True,
                    )
                es = es_pool.tile([P, 2, QC], bf16, tag="es", name="es")
                nc.scalar.activation(
                    es[:].rearrange("p a b -> p (a b)"),
                    s[:].rearrange("p a b -> p (a b)"),
                    Exp, scale=0.125)
                return es

            def emit_pv(j, qc, ktn, es, pva, pvb):
                vv = v_sb[j]
                first = ktn == 0
                last = ktn == NKT - 1
                nc.tensor.matmul(pva[:], vv[:, ktn, 0, :], es[:, 0, :],
                                 start=first, stop=last)
                nc.tensor.matmul(pvb[:], vv[:, ktn, 1, :], es[:, 1, :],
                                 start=first, stop=last)

            for j in range(NPAIR):
                for qc in range(NQC):
                    q0 = QC * qc
                    pva = pv_pool.tile([HD + 1, QC], fp32, tag="pva")
                    pvb = pv_pool.tile([HD + 1, QC], fp32, tag="pvb")
                    if j == 0 and qc == 0:
                        # first iteration: pipeline the V projection with
                        # the attention — PV for k-tile kt only needs V
                        # t-tile kt, so V tiles are produced just-in-time
                        # while the scalar engine works through the exps
                        for ktn in range(NKT):
                            es = emit_scores_exp(j, qc, ktn)
                            emit_v_proj(ktn, ktn + 1)
                            emit_pv(j, qc, ktn, es, pva, pvb)
                    else:
                        for ktn in range(NKT):
                            # keep the PE warm through ACT-paced stretches:
                            # the last pair has no projection filler left,
                            # so issue tiny throwaway matmuls (HAM
                            # re-throttles the PE clock after ~3.4us of
                            # contiguous idle)
                            if j == NPAIR - 1 and ktn % 2 == 0:
                                dm = qkv_pool.tile([P, 256], fp32,
                                                   tag="qkv", name="warmmm")
                                nc.tensor.matmul(
                                    dm[:], wq_sb[:, 0, 0:P],
                                    xt_sb[:, 0, 0:256],
                                    start=True, stop=True)
                            es = emit_scores_exp(j, qc, ktn)
                            emit_pv(j, qc, ktn, es, pva, pvb)
                    # normalize: row HD of pv holds Z = sum_k exp(s/8).
                    # Copy psum->sbuf first so the PV banks free up fast
                    # (the recip/broadcast chain is slow but off-critical).
                    pvcs = []
                    for hp, pv_t in ((0, pva), (1, pvb)):
                        pvc = norm_pool.tile([HD + 1, QC], fp32,
                                             tag=f"pvc{hp}", name=f"pvc{hp}")
                        nc.vector.tensor_copy(pvc[:], pv_t[:])
                        pvcs.append(pvc)
                    for hp in (0, 1):
                        pvc = pvcs[hp]
                        nc.sync.dma_start(out=zcol[hp][:],
                                          in_=pvc[HD:HD + 1, :])
                        nc.vector.reciprocal(zcol[hp][:], zcol[hp][:])
                        nc.sync.dma_start(out=rz0[hp][:], in_=zcol[hp][:])
                        nc.gpsimd.partition_broadcast(rzb[hp][:], rz0[hp][:])
                        st = stage_pool.tile([HD, QC], fp32, tag=f"st{hp}",
                                             name=f"st{hp}")
                        nc.vector.tensor_mul(st[:], pvc[0:HD, :],
                                             rzb[hp][:])
                        nc.sync.dma_start(out=o[2 * j + hp, :, q0:q0 + QC],
                                          in_=st[:])
                    # feed the PE pipeline with projection filler, spread
                    # over every iteration: next pair's K^T chunk-by-chunk,
                    # this pair's remaining Q^T chunks just before use, and
                    # next pair's first Q^T chunk at the boundary
                    for tcn in range(qc * NTC // NQC, (qc + 1) * NTC // NQC):
                        if j + 1 < NPAIR:
                            emit_qk_chunk(j + 1, "k", tcn)
                    nxt = (qc + 1) * NTC // NQC
                    if nxt < NTC:
                        emit_qk_chunk(j, "q", nxt)
                    elif j + 1 < NPAIR:
                        emit_qk_chunk(j + 1, "q", 0)

    nc.compile()
    return nc


def _prep_inputs(x, Wq, bq, Wk, bk, Wv, bv):
    """Host-side shard + layout prep. Returns per-core input dicts."""
    in_maps = []
    xt_cache = {}
    w_cache = {}
    for c in range(N_CORES):
        b, g = c // G, c % G
        if b not in xt_cache:
            xtb = np.ascontiguousarray(x[b].T).astype(BF16)      # [D, T]
            xt_cache[b] = np.ascontiguousarray(
                xtb.reshape(DC, P, T).transpose(1, 0, 2))        # [P, DC, T]
        if g not in w_cache:
            def _w(W):
                Wg = W[:, F * g:F * (g + 1)].astype(BF16)        # [D, F]
                return np.ascontiguousarray(
                    Wg.reshape(DC, P, F).transpose(1, 0, 2))     # [P, DC, F]
            bqg = bq[F * g:F * (g + 1)].astype(np.float32)
            bkg = bk[F * g:F * (g + 1)].astype(np.float32)
            bvg = bv[F * g:F * (g + 1)].astype(np.float32)
            w_cache[g] = {
                "wq": _w(Wq), "wk": _w(Wk), "wv": _w(Wv),
                # [P, NPAIR]: bias for feature 128*j + p
                "bq": np.ascontiguousarray(bqg.reshape(NPAIR, P).T),
                "bk": np.ascontiguousarray(bkg.reshape(NPAIR, P).T),
                # [P, F]: broadcast along partitions
                "bv": np.ascontiguousarray(
                    np.broadcast_to(bvg[None, :], (P, F))),
            }
        in_maps.append({"xt": xt_cache[b], **w_cache[g]})
    return in_maps


def _run(in_maps, trace_dir=None, trace_cores=None):
    from concourse.bass_utils import run_bass_kernel_spmd

    global _compiled
    if _compiled is None:
        _compiled = _build()
    nc = _compiled

    if trace_dir is not None:
        from trn_agent_boot.trn_boot import _ntff_profile_via_ctypes
        hook = _ntff_profile_via_ctypes("/opt/axon/libaxon_pjrt.so")
        with hook(trace_dir, trace_cores):
            res = run_bass_kernel_spmd(nc, in_maps,
                                       core_ids=list(range(N_CORES)))
    else:
        res = run_bass_kernel_spmd(nc, in_maps, core_ids=list(range(N_CORES)))
    return res


def kernel(x, Wq, bq, Wk, bk, Wv, bv, _trace_dir=None, _trace_cores=None):
    x = np.asarray(x, dtype=np.float32)
    in_maps = _prep_inputs(x, np.asarray(Wq), np.asarray(bq), np.asarray(Wk),
                           np.asarray(bk), np.asarray(Wv), np.asarray(bv))
    res = _run(in_maps, _trace_dir, _trace_cores)
    out = np.empty((B, T, D), np.float32)
    for c in range(N_CORES):
        b, g = c // G, c % G
        oc = np.asarray(res.results[c]["o"])          # [HPC, HD, T]
        out[b, :, F * g:F * (g + 1)] = (
            oc.transpose(2, 0, 1).reshape(T, F))
    return out
